# revision 1
# baseline (speedup 1.0000x reference)
"""BitAttention (ternary-weight attention with int4/topk-int8 activation quant)
on 8 Trainium2 NeuronCores.

Sharding: tensor-parallel over heads for qkv-proj + SDPA (heads/8 per core),
AllToAll re-shard to token-parallel for the topk+int8 o-projection.

Numerics: quantized values are exact small integers, so qkv/o projections run
as exact integer arithmetic in bf16 matmuls (fp32 PSUM accumulate). Attention
(rope'd q/k real-valued) runs in fp32 matmuls. Softmax exp on ACT. Top-k
per-row threshold found by binary search on the |value| axis; per-token scales
folded into rope tables / exp bias / output scaling.
"""
import math
import numpy as np
import ml_dtypes

# ---------------------------------------------------------------------------
# TileContext patches for this walrus build (single sem-wait per instruction).
# ---------------------------------------------------------------------------
import re as _re
import concourse.mybir as mybir
import concourse.bass as bass
import concourse.tile as tile
from concourse.tile import TileContext, ScopedClock, VectorClock
from concourse.bass_utils import run_bass_kernel_spmd

_carrier_seq = [0]
_orig_add_instruction = TileContext._add_instruction


def _patched_add_instruction(self, inst):
    si = inst.sync_info
    if si is not None and si.on_wait is not None and len(si.on_wait) > 1:
        waits = list(si.on_wait)
        for w in waits[:-1]:
            _carrier_seq[0] += 1
            carrier = mybir.InstEventSemaphore(
                name=f"waitc_{_carrier_seq[0]}_{inst.name}",
                engine=inst.engine,
                ins=[],
                outs=[],
                sync_info=mybir.SyncInfo(on_wait=[w], on_update=[]),
            )
            _orig_add_instruction(self, carrier)
        si.on_wait = [waits[-1]]
        inst.sync_info = si
    _orig_add_instruction(self, inst)


def _clock_ticks(clock):
    m = _re.match(r"VectorClock\((\[.*\])\)", repr(clock))
    return eval(m.group(1))


def _patched_drain_and_barrier(self, tick_clock, wait_clock):
    nc = self.nc
    ticks = _clock_ticks(tick_clock.global_clock)
    n = len(ticks)
    for i, t in enumerate(ticks):
        if t > 0:
            d = nc.sync.drain()
            vci = VectorClock([t if j == i else 0 for j in range(n)])
            wait_clock.add_sem_waits(d.ins, ScopedClock({None: vci}))
    nc.sync.drain()
    nc.all_engine_barrier()
    assert self.sems is not None
    popped = nc._tile_sem_poison_stack.pop()
    assert popped is self._sem_poison
    nc.clear_and_free_semaphores(list(self.sems.allocated().values()))
    nc.all_engine_barrier()


TileContext._add_instruction = _patched_add_instruction
TileContext._drain_and_barrier = _patched_drain_and_barrier

# ---------------------------------------------------------------------------

F32 = mybir.dt.float32
BF16 = mybir.dt.bfloat16
AF = mybir.ActivationFunctionType
ALU = mybir.AluOpType
AX = mybir.AxisListType
MAGIC = 1.5 * 2.0 ** 23
EPS = 1e-5
THETA = 10000.0
TOPK_RATIO = 0.55
NCORES = 8


class Cfg:
    def __init__(self, B=2, T=2048, D=2048, H=16, HD=128, chunk=256, qchunk=256,
                 search_iters=26, no_collectives=False, stop_after=''):
        self.B, self.T, self.D, self.H, self.HD = B, T, D, H, HD
        self.NT = B * T
        self.HPC = H // NCORES            # heads per core
        self.FS = self.HPC * HD           # feature slice per core
        self.chunk = chunk                # phase-A token chunk
        self.qchunk = qchunk              # attention q chunk
        self.TPC = self.NT // NCORES      # tokens per core in phase C
        self.K = max(1, int(TOPK_RATIO * D))
        self.search_iters = search_iters
        self.no_collectives = no_collectives
        self.stop_after = stop_after
        assert self.NT % 128 == 0 and D % 512 == 0 and HD % 2 == 0
        assert T % qchunk == 0 and self.NT % chunk == 0 and chunk % 128 == 0
        assert self.TPC % 128 == 0 and HD <= 128 and self.FS % 128 == 0
        assert D == H * HD


def rope_tables(cfg):
    hd, T = cfg.HD, cfg.T
    inv = 1.0 / THETA ** (np.arange(0, hd, 2, dtype=np.float32) / hd)
    freqs = np.arange(T, dtype=np.float32)[:, None] * inv[None, :]
    emb = np.concatenate([freqs, freqs], axis=1)          # (T, hd)
    cos = np.cos(emb).astype(np.float32)
    sin = np.sin(emb).astype(np.float32)
    cosT = np.concatenate([cos] * cfg.B, 0).T.copy()      # (hd, NT)
    sinT = np.concatenate([sin] * cfg.B, 0).T.copy()
    sin_pm = sinT.copy()
    sin_pm[: hd // 2] = -sin_pm[: hd // 2]                # rotate-half signs
    return np.ascontiguousarray(cosT), np.ascontiguousarray(sin_pm)


def build(cfg: Cfg):
    nc = bass.Bass("TRN2", target_bir_lowering=False, debug=False,
                   num_devices=NCORES)
    NT, D, HD, FS, TPC = cfg.NT, cfg.D, cfg.HD, cfg.FS, cfg.TPC

    x_d = nc.dram_tensor("x", [NT, D], F32, kind="ExternalInput")
    wqT_d = nc.dram_tensor("wqT", [D, FS], F32, kind="ExternalInput")
    wkT_d = nc.dram_tensor("wkT", [D, FS], F32, kind="ExternalInput")
    wvT_d = nc.dram_tensor("wvT", [D, FS], F32, kind="ExternalInput")
    woT_d = nc.dram_tensor("woT", [D, D], F32, kind="ExternalInput")
    cos_d = nc.dram_tensor("cosT", [HD, NT], F32, kind="ExternalInput")
    sin_d = nc.dram_tensor("sinpmT", [HD, NT], F32, kind="ExternalInput")
    idf_d = nc.dram_tensor("idf", [128, 128], F32, kind="ExternalInput")
    idb_d = nc.dram_tensor("idb", [128, 128], BF16, kind="ExternalInput")
    y_d = nc.dram_tensor("y", [TPC, D], F32, kind="ExternalOutput")

    with TileContext(nc, pool_alloc_mode="queue") as tc:
        _body(nc, tc, cfg, x_d, wqT_d, wkT_d, wvT_d, woT_d, cos_d, sin_d,
              idf_d, idb_d, y_d)
    return nc


def _body(nc, tc, cfg, x_d, wqT_d, wkT_d, wvT_d, woT_d, cos_d, sin_d,
          idf_d, idb_d, y_d):
    NT, D, HD, HPC, FS = cfg.NT, cfg.D, cfg.HD, cfg.HPC, cfg.FS
    NTT, NDT = NT // 128, D // 128
    CH = cfg.chunk
    NCH, CTT = NT // CH, CH // 128
    QC, KT, NQC = cfg.qchunk, cfg.T // 128, cfg.T // cfg.qchunk
    TPC, FTQ = cfg.TPC, FS // 128
    SQRT_SCALE = float(1.0 / math.sqrt(HD))
    HH = HD // 2

    with tc.tile_pool(name="persist", bufs=1) as pp, \
         tc.tile_pool(name="dram", bufs=1, space="DRAM") as dramp:
        idf = pp.tile([128, 128], F32)
        nc.sync.dma_start(idf[:], idf_d[:])
        idb = pp.tile([128, 128], BF16)
        nc.sync.dma_start(idb[:], idb_d[:])
        ones_col = pp.tile([128, 1], F32)        # partition sums (lhsT)
        nc.gpsimd.memset(ones_col[:], 1.0)
        ones_row = pp.tile([1, 128], F32)        # partition broadcast (lhsT)
        nc.gpsimd.memset(ones_row[:], 1.0)
        # per token-tile columns (tokens on partitions)
        inv_sx = pp.tile([128, NTT], F32)
        ln_sv = pp.tile([128, NTT], F32)
        rinv_sv = pp.tile([128, NTT], F32)
        ws_s = pp.tile([128, 4], F32)   # bcast weight scales s_w (q,k,v,o)
        ws_r = pp.tile([128, 4], F32)   # bcast 1/(s_w+eps)
        sob = pp.tile([128, 2], F32)    # bcast (s_wo, 1/(s_wo+eps))
        a2a_in = dramp.tile([NT, FS], F32)
        a2a_out = dramp.tile([NT, FS], F32)

        with tc.tile_pool(name="pAB", bufs=1) as pab:
            # roped q/k [FS, NT] fp32; integer v in token layout (fp32 ints)
            qT = [pab.tile([128, NT], F32, tag=f"qT{i}", name=f"qT{i}") for i in range(FTQ)]
            kT = [pab.tile([128, NT], F32, tag=f"kT{i}", name=f"kT{i}") for i in range(FTQ)]
            vtok = [pab.tile([128, FS], F32, tag=f"vtok{i}", name=f"vtok{i}") for i in range(NTT)]

            with tc.tile_pool(name="pW", bufs=1) as pw:
                # ternarized weight slices (bf16 ints), persist through phase A
                wqt = [pw.tile([128, FS], BF16, tag=f"wqt{i}", name=f"wqt{i}") for i in range(NDT)]
                wkt = [pw.tile([128, FS], BF16, tag=f"wkt{i}", name=f"wkt{i}") for i in range(NDT)]
                wvt = [pw.tile([128, FS], BF16, tag=f"wvt{i}", name=f"wvt{i}") for i in range(NDT)]
                _phase_w(nc, tc, cfg, dramp, wqT_d, wkT_d, wvT_d, woT_d,
                         ones_col, ones_row, ws_s, ws_r, wqt, wkt, wvt)
                if cfg.stop_after == 'W':
                    return
                _phase_a(nc, tc, cfg, x_d, cos_d, sin_d, idf, idb, ones_row,
                         ws_s, inv_sx, ln_sv, rinv_sv, wqt, wkt, wvt,
                         qT, kT, vtok)
            if cfg.stop_after == 'A':
                return
            wom = _WoMean(nc, tc, cfg, woT_d, ones_col, ones_row, sob)
            _phase_b(nc, tc, cfg, idf, ones_col, ones_row, ln_sv, rinv_sv,
                     qT, kT, vtok, a2a_in, wom)
            wom.finish()
        if cfg.stop_after == 'B':
            return

        if cfg.no_collectives:
            nc.sync.dma_start(a2a_out[:], a2a_in[:])
        else:
            nc.gpsimd.collective_compute(
                "AllToAll", ALU.bypass, replica_groups=[list(range(NCORES))],
                ins=[a2a_in[:].opt()], outs=[a2a_out[:].opt()])
        _phase_c(nc, tc, cfg, woT_d, idb, ws_s, ws_r, ones_col, ones_row, sob, a2a_out, y_d)


def _phase_w(nc, tc, cfg, dramp, wqT_d, wkT_d, wvT_d, woT_d, ones_col,
             ones_row, ws_s, ws_r, wqt, wkt, wvt):
    D, FS = cfg.D, cfg.FS
    NDT = D // 128
    with tc.tile_pool(name="ph_w", bufs=1) as wp, \
         tc.tile_pool(name="ph_w_ps", bufs=2, space="PSUM") as wps, \
         tc.tile_pool(name="ph_w1", bufs=1) as wp1:
        partials = wp1.tile([1, 4], F32)
        wraw = {}
        for j, wd in enumerate([wqT_d, wkT_d, wvT_d]):
            acc = wp1.tile([128, 1], F32, tag=f"wacc{j}", name=f"wacc{j}")
            nc.gpsimd.memset(acc[:], 0.0)
            for dt in range(NDT):
                t = wp.tile([128, FS], F32, tag=f"wld_{j}_{dt}",
                            name=f"wld_{j}_{dt}")
                nc.sync.dma_start(t[:], wd[dt * 128:(dt + 1) * 128, :])
                wraw[(j, dt)] = t
                r = wp1.tile([128, 1], F32, tag="wred")
                nc.vector.tensor_reduce(r[:], t[:], axis=AX.X, op=ALU.add,
                                        apply_absolute_value=True)
                nc.vector.tensor_tensor(acc[:], acc[:], r[:], op=ALU.add)
            ps = wps.tile([1, 1], F32, tag="w_ps1")
            nc.tensor.matmul(ps[:], acc[:], ones_col[:, 0:1], start=True,
                             stop=True)
            nc.vector.tensor_copy(partials[:, j:j + 1], ps[:])
        nc.gpsimd.memset(partials[:, 3:4], 0.0)
        ar_in = dramp.tile([1, 4], F32)
        ar_out = dramp.tile([1, 4], F32, addr_space="Shared")
        nc.sync.dma_start(ar_in[:], partials[:])
        if cfg.no_collectives:
            nc.sync.dma_start(ar_out[:], ar_in[:])
        else:
            nc.gpsimd.collective_compute(
                "AllReduce", ALU.add, replica_groups=[list(range(NCORES))],
                ins=[ar_in[:].opt()], outs=[ar_out[:].opt()])
        sums = wp1.tile([1, 4], F32)
        nc.sync.dma_start(sums[:], ar_out[:])
        s_row = wp1.tile([1, 4], F32)
        nc.vector.tensor_scalar(s_row[:], sums[:], 1.0 / (float(D) * float(D)),
                                None, op0=ALU.mult)
        r_row = wp1.tile([1, 4], F32)
        nc.vector.tensor_scalar(r_row[:], s_row[:], EPS, None, op0=ALU.add)
        nc.vector.reciprocal(r_row[:], r_row[:])
        ps_b = wps.tile([128, 4], F32, tag="w_psb")
        nc.tensor.matmul(ps_b[:], ones_row[:], s_row[:], start=True, stop=True)
        nc.scalar.copy(ws_s[:], ps_b[:])
        ps_b2 = wps.tile([128, 4], F32, tag="w_psb")
        nc.tensor.matmul(ps_b2[:], ones_row[:], r_row[:], start=True,
                         stop=True)
        nc.scalar.copy(ws_r[:], ps_b2[:])
        # ternarize in place from the resident raw tiles
        for j, dst in enumerate([wqt, wkt, wvt]):
            for dt in range(NDT):
                t = wraw[(j, dt)]
                nc.vector.tensor_scalar(t[:], t[:], ws_r[:, j:j + 1], MAGIC,
                                        op0=ALU.mult, op1=ALU.add)
                nc.vector.tensor_scalar(t[:], t[:], MAGIC, -1.0,
                                        op0=ALU.subtract, op1=ALU.max)
                nc.vector.tensor_scalar(dst[dt][:], t[:], 1.0, None,
                                        op0=ALU.min)


def _phase_a(nc, tc, cfg, x_d, cos_d, sin_d, idf, idb, ones_row, ws_s,
             inv_sx, ln_sv, rinv_sv, wqt, wkt, wvt, qT, kT, vtok):
    D, HD, FS = cfg.D, cfg.HD, cfg.FS
    NDT = D // 128
    CH = cfg.chunk
    NCH, CTT = cfg.NT // CH, CH // 128
    FTQ = FS // 128
    HH = HD // 2
    with tc.tile_pool(name="ph_a", bufs=2) as ap, \
         tc.tile_pool(name="ph_a3", bufs=3) as ap3, \
         tc.tile_pool(name="ph_a_ps", bufs=3, space="PSUM") as aps, \
         tc.tile_pool(name="ph_a_ps3", bufs=4, space="PSUM") as aps3:
        for ch in range(NCH):
            t0 = ch * CH
            xq = [None] * CTT
            for j in range(CTT):
                tt = t0 // 128 + j
                xt = ap.tile([128, D], F32, tag="xload")
                nc.sync.dma_start(xt[:], x_d[tt * 128:(tt + 1) * 128, :])
                m = ap.tile([128, 1], F32, tag="xm")
                nc.vector.tensor_reduce(m[:], xt[:], axis=AX.X, op=ALU.max,
                                        apply_absolute_value=True)
                nc.vector.tensor_scalar(m[:], m[:], EPS, None, op0=ALU.max)
                nc.vector.tensor_scalar(inv_sx[:, tt:tt + 1], m[:], 1.0 / 7.0,
                                        None, op0=ALU.mult)
                sx = ap.tile([128, 1], F32, tag="xs")
                nc.vector.reciprocal(sx[:], m[:])
                nc.vector.tensor_scalar(sx[:], sx[:], 7.0, None, op0=ALU.mult)
                sv = ap.tile([128, 1], F32, tag="xsv")
                nc.vector.tensor_tensor(sv[:], inv_sx[:, tt:tt + 1],
                                        ws_s[:, 2:3], op=ALU.mult)
                nc.scalar.activation(ln_sv[:, tt:tt + 1], sv[:], AF.Ln)
                nc.vector.reciprocal(rinv_sv[:, tt:tt + 1], sv[:])
                xqj = ap.tile([128, D], BF16, tag="xq")
                tmp = ap.tile([128, D], F32, tag="xtmp")
                nc.vector.tensor_scalar(tmp[:], xt[:], sx[:], MAGIC,
                                        op0=ALU.mult, op1=ALU.add)
                nc.vector.tensor_scalar(xqj[:], tmp[:], MAGIC, None,
                                        op0=ALU.subtract)
                xq[j] = xqj
            # transpose xq -> xqT tiles [128 d, CH] (bf16 ints)
            xqT = [None] * NDT
            for dt in range(NDT):
                pst = aps.tile([128, CH], BF16, tag="ps_misc")
                for j in range(CTT):
                    nc.tensor.transpose(pst[:, j * 128:(j + 1) * 128],
                                        xq[j][:, dt * 128:(dt + 1) * 128],
                                        idb[:])
                xqT[dt] = ap.tile([128, CH], BF16, tag=f"xqT{dt}", name=f"xqT{dt}")
                nc.scalar.copy(xqT[dt][:], pst[:])
            # scaled rope tables for this chunk
            cos_c = ap.tile([HD, CH], F32, tag="cos_c")
            nc.sync.dma_start(cos_c[:], cos_d[:, t0:t0 + CH])
            sin_c = ap.tile([HD, CH], F32, tag="sin_c")
            nc.sync.dma_start(sin_c[:], sin_d[:, t0:t0 + CH])
            tabs = {}
            for (wj, nm) in ((0, 'q'), (1, 'k')):
                colp = ap.tile([128, CTT], F32, tag="colp")
                nc.vector.tensor_scalar(colp[:],
                                        inv_sx[:, t0 // 128:t0 // 128 + CTT],
                                        ws_s[:, wj:wj + 1], None, op0=ALU.mult)
                pst = aps.tile([128, 128], F32, tag="ps_misc")
                nc.tensor.transpose(pst[:CTT, :], colp[:], idf[:])
                srow_t = ap.tile([CTT, 128], F32, tag="srowt")
                nc.scalar.copy(srow_t[:], pst[:CTT, :])
                srow = ap.tile([1, CH], F32, tag="srow")
                nc.sync.dma_start(srow[:], srow_t[:])
                psb = aps.tile([HD, CH], F32, tag="ps_misc")
                nc.tensor.matmul(psb[:], ones_row[:, :HD], srow[:],
                                 start=True, stop=True)
                sb = ap.tile([HD, CH], F32, tag=f"sb_{nm}", name=f"sb_{nm}")
                nc.scalar.copy(sb[:], psb[:])
                tc_t = ap.tile([HD, CH], F32, tag=f"tc_{nm}", name=f"tc_{nm}")
                nc.vector.tensor_tensor(tc_t[:], cos_c[:], sb[:], op=ALU.mult)
                ts_t = ap.tile([HD, CH], F32, tag=f"ts_{nm}", name=f"ts_{nm}")
                nc.vector.tensor_tensor(ts_t[:], sin_c[:], sb[:], op=ALU.mult)
                tabs[nm] = (tc_t, ts_t)
            # projections + rope drains
            for wt, nm, dstT in ((wqt, 'q', qT), (wkt, 'k', kT)):
                table_c, table_s = tabs[nm]
                for ft in range(FTQ):
                    ps = aps3.tile([128, CH], F32, tag="ps_proj")
                    for dt in range(NDT):
                        nc.tensor.matmul(ps[:],
                                         wt[dt][:, ft * 128:(ft + 1) * 128],
                                         xqT[dt][:], start=(dt == 0),
                                         stop=(dt == NDT - 1))
                    dtile = dstT[ft]
                    for hb in range(128 // HD):
                        fo = hb * HD
                        t1 = ap3.tile([HD, CH], F32, tag="ropet1")
                        nc.vector.tensor_tensor(t1[:], ps[fo:fo + HD, :],
                                                table_c[:], op=ALU.mult)
                        t2 = ap3.tile([HD, CH], F32, tag="ropet2")
                        nc.vector.tensor_tensor(t2[:HH, :],
                                                ps[fo + HH:fo + HD, :],
                                                table_s[:HH, :], op=ALU.mult)
                        nc.vector.tensor_tensor(t2[HH:, :], ps[fo:fo + HH, :],
                                                table_s[HH:, :], op=ALU.mult)
                        nc.vector.tensor_tensor(dtile[fo:fo + HD, t0:t0 + CH],
                                                t1[:], t2[:], op=ALU.add)
            # v: integer result, drain fp32, transpose to token layout
            for ft in range(FTQ):
                ps = aps3.tile([128, CH], F32, tag="ps_proj")
                for dt in range(NDT):
                    nc.tensor.matmul(ps[:], wvt[dt][:, ft * 128:(ft + 1) * 128],
                                     xqT[dt][:], start=(dt == 0),
                                     stop=(dt == NDT - 1))
                vtmp = ap3.tile([128, CH], F32, tag="vtmp")
                nc.scalar.copy(vtmp[:], ps[:])
                for j in range(CTT):
                    tt = t0 // 128 + j
                    pst = aps.tile([128, 128], F32, tag="ps_misc")
                    nc.tensor.transpose(pst[:], vtmp[:, j * 128:(j + 1) * 128],
                                        idf[:])
                    nc.scalar.copy(vtok[tt][:, ft * 128:(ft + 1) * 128],
                                   pst[:])


class _WoMean:
    """mean|wo| pass, emitted one tile per attention iteration so the DVE/DMA
    work interleaves with phase B instead of serializing before it."""

    def __init__(self, nc, tc, cfg, woT_d, ones_col, ones_row, sob):
        self.nc, self.tc, self.cfg = nc, tc, cfg
        self.woT_d, self.ones_col, self.ones_row, self.sob = (
            woT_d, ones_col, ones_row, sob)
        self.NDT = cfg.D // 128
        self._cms = [tc.tile_pool(name="pwo", bufs=2),
                     tc.tile_pool(name="pwo1", bufs=1),
                     tc.tile_pool(name="pwo_ps", bufs=1, space="PSUM")]
        self.wp = self._cms[0].__enter__()
        self.wp1 = self._cms[1].__enter__()
        self.wps = self._cms[2].__enter__()
        self.acc = self.wp1.tile([128, 1], F32, name="wo_acc")
        nc.gpsimd.memset(self.acc[:], 0.0)
        self.done = 0

    def step(self):
        if self.done >= self.NDT:
            return
        nc, D = self.nc, self.cfg.D
        dt = self.done
        self.done += 1
        t = self.wp.tile([128, D], F32, tag="wo_ld", name="wo_ld")
        nc.sync.dma_start(t[:], self.woT_d[dt * 128:(dt + 1) * 128, :])
        r = self.wp1.tile([128, 1], F32, tag="wo_red", name="wo_red")
        nc.vector.tensor_reduce(r[:], t[:], axis=AX.X, op=ALU.add,
                                apply_absolute_value=True)
        nc.vector.tensor_tensor(self.acc[:], self.acc[:], r[:], op=ALU.add)

    def finish(self):
        while self.done < self.NDT:
            self.step()
        nc, D = self.nc, self.cfg.D
        pss = self.wps.tile([1, 1], F32, tag="wo_ps", name="wo_ps")
        nc.tensor.matmul(pss[:], self.acc[:], self.ones_col[:, 0:1],
                         start=True, stop=True)
        so_s = self.wp1.tile([1, 1], F32, name="so_s")
        nc.vector.tensor_scalar(so_s[:], pss[:], 1.0 / (float(D) * float(D)),
                                None, op0=ALU.mult)
        so_r = self.wp1.tile([1, 1], F32, name="so_r")
        nc.vector.tensor_scalar(so_r[:], so_s[:], EPS, None, op0=ALU.add)
        nc.vector.reciprocal(so_r[:], so_r[:])
        sr2 = self.wp1.tile([1, 2], F32, name="sr2")
        nc.vector.tensor_copy(sr2[:, 0:1], so_s[:])
        nc.vector.tensor_copy(sr2[:, 1:2], so_r[:])
        psb = self.wps.tile([128, 2], F32, tag="wo_ps", name="wo_psb")
        nc.tensor.matmul(psb[:], self.ones_row[:], sr2[:], start=True,
                         stop=True)
        nc.scalar.copy(self.sob[:], psb[:])
        for cm in reversed(self._cms):
            cm.__exit__(None, None, None)


def _phase_b(nc, tc, cfg, idf, ones_col, ones_row, ln_sv, rinv_sv,
             qT, kT, vtok, a2a_in, wom=None):
    HD, HPC = cfg.HD, cfg.HPC
    QC, KT, NQC = cfg.qchunk, cfg.T // 128, cfg.T // cfg.qchunk
    with tc.tile_pool(name="ph_b_p", bufs=2) as bp, \
         tc.tile_pool(name="ph_b_pt", bufs=2) as bpt, \
         tc.tile_pool(name="ph_b_ps_sc", bufs=3, space="PSUM") as ps_sc, \
         tc.tile_pool(name="ph_b_ps_o", bufs=2, space="PSUM") as ps_o, \
         tc.tile_pool(name="ph_b_ps_m", bufs=2, space="PSUM") as ps_m:
        for b in range(cfg.B):
            for hh in range(HPC):
                fo = hh * HD
                ftile, fin = fo // 128, fo % 128
                for qc in range(NQC):
                    q0 = b * cfg.T + qc * QC
                    pT = [bpt.tile([128, QC], F32, tag=f"pT{i}", name=f"pT{i}")
                          for i in range(KT)]
                    den = bp.tile([128, QC], F32, tag="den")
                    outp = ps_o.tile([HD, QC], F32, tag="outp")
                    if wom is not None:
                        wom.step()
                    for kt in range(KT):
                        ktt = (b * cfg.T) // 128 + kt
                        k0 = b * cfg.T + kt * 128
                        ssc = ps_sc.tile([128, QC], F32, tag="ssc")
                        nc.tensor.matmul(
                            ssc[:], kT[ftile][fin:fin + HD, k0:k0 + 128],
                            qT[ftile][fin:fin + HD, q0:q0 + QC],
                            start=True, stop=True)
                        nc.scalar.activation(pT[kt][:], ssc[:], AF.Exp,
                                             bias=ln_sv[:, ktt:ktt + 1],
                                             scale=SQRT_SCALE_OF(cfg))
                        if kt == 0:
                            nc.vector.tensor_scalar(den[:], pT[kt][:],
                                                    rinv_sv[:, ktt:ktt + 1],
                                                    None, op0=ALU.mult)
                        else:
                            nc.vector.scalar_tensor_tensor(
                                den[:], in0=pT[kt][:],
                                scalar=rinv_sv[:, ktt:ktt + 1], in1=den[:],
                                op0=ALU.mult, op1=ALU.add)
                        nc.tensor.matmul(outp[:], vtok[ktt][:, fo:fo + HD],
                                         pT[kt][:], start=(kt == 0),
                                         stop=(kt == KT - 1))
                    dps = ps_m.tile([1, QC], F32, tag="ps_misc")
                    nc.tensor.matmul(dps[:], ones_col[:], den[:], start=True,
                                     stop=True)
                    drow = bp.tile([1, QC], F32, tag="drow")
                    nc.vector.reciprocal(drow[:], dps[:])
                    rdb = ps_m.tile([HD, QC], F32, tag="ps_misc")
                    nc.tensor.matmul(rdb[:], ones_row[:, :HD], drow[:],
                                     start=True, stop=True)
                    osb = bp.tile([HD, QC], F32, tag="osb")
                    nc.scalar.copy(osb[:], outp[:])
                    nc.vector.tensor_tensor(osb[:], osb[:], rdb[:],
                                            op=ALU.mult)
                    for j in range(QC // 128):
                        pst = ps_m.tile([128, HD], F32, tag="ps_misc")
                        nc.tensor.transpose(pst[:],
                                            osb[:, j * 128:(j + 1) * 128],
                                            idf[:])
                        stg = bp.tile([128, HD], F32, tag="stg")
                        nc.scalar.copy(stg[:], pst[:])
                        r0 = q0 + j * 128
                        nc.sync.dma_start(a2a_in[r0:r0 + 128, fo:fo + HD],
                                          stg[:])


def SQRT_SCALE_OF(cfg):
    return float(1.0 / math.sqrt(cfg.HD))


def _phase_c(nc, tc, cfg, woT_d, idb, ws_s, ws_r, ones_col, ones_row, sob, a2a_out, y_d):
    D, TPC = cfg.D, cfg.TPC
    NDT = D // 128
    NTC = TPC // 128
    NFC = D // 512
    with tc.tile_pool(name="pc0", bufs=1) as pc0:
        m8 = pc0.tile([128, NTC], F32)
        lo = pc0.tile([128, NTC], F32)
        s8 = pc0.tile([128, NTC], F32)
        x8 = [pc0.tile([128, D], BF16, tag=f"x8_{j}", name=f"x8_{j}")
              for j in range(NTC)]
        # --- C1: load, abs, threshold search, int8 quant + mask ---
        with tc.tile_pool(name="pc1", bufs=1) as cp1, \
             tc.tile_pool(name="pc1w", bufs=3) as cpw:
            a2a_v = a2a_out[:].rearrange("(s t) f -> t s f", s=NCORES)
            at, absa = [], []
            for j in range(NTC):
                t = cp1.tile([128, D], F32, tag=f"at{j}", name=f"at{j}")
                nc.sync.dma_start(t[:].rearrange("p (s f) -> p s f", s=NCORES),
                                  a2a_v[j * 128:(j + 1) * 128])
                at.append(t)
                ab = cp1.tile([128, D], F32, tag=f"ab{j}", name=f"ab{j}")
                nc.scalar.activation(ab[:], t[:], AF.Abs)
                absa.append(ab)
                nc.vector.tensor_reduce(m8[:, j:j + 1], ab[:], axis=AX.X,
                                        op=ALU.max)
            nc.vector.tensor_scalar(m8[:], m8[:], EPS, None, op0=ALU.max)
            # binary search for the k-th largest |a| per row
            nc.gpsimd.memset(lo[:], 0.0)
            hi = cp1.tile([128, NTC], F32)
            nc.vector.tensor_scalar(hi[:], m8[:], 1.0001, None, op0=ALU.mult)
            mid = cp1.tile([128, NTC], F32)
            nmid = cp1.tile([128, NTC], F32)
            cnt = cp1.tile([128, NTC], F32)
            ge = cp1.tile([128, NTC], F32)
            dif = cp1.tile([128, NTC], F32)
            junk = cp1.tile([128, D], F32)
            junka = cp1.tile([128, D], F32)
            # first iters: upper half of the token tiles counted on ACT via
            # Sign+accum (acc = #above - #below); later iters all on DVE
            # (exact >= semantics near convergence).
            nh = NTC // 2
            act_iters = max(0, cfg.search_iters - 10) if nh else 0
            for it in range(cfg.search_iters):
                nc.vector.tensor_tensor(mid[:], lo[:], hi[:], op=ALU.add)
                nc.vector.tensor_scalar(mid[:], mid[:], 0.5, None, op0=ALU.mult)
                use_act = it < act_iters
                if use_act:
                    nc.vector.tensor_scalar(nmid[:], mid[:], -1.0, None,
                                            op0=ALU.mult)
                for j in range(NTC):
                    if use_act and j >= NTC - nh:
                        nc.scalar.activation(junka[:], absa[j][:], AF.Sign,
                                             bias=nmid[:, j:j + 1],
                                             accum_out=cnt[:, j:j + 1])
                    else:
                        nc.vector.tensor_scalar(junk[:], absa[j][:],
                                                mid[:, j:j + 1], None,
                                                op0=ALU.is_ge, op1=ALU.add,
                                                accum_out=cnt[:, j:j + 1])
                if use_act:
                    nc.vector.tensor_scalar(ge[:, :NTC - nh],
                                            cnt[:, :NTC - nh], float(cfg.K),
                                            None, op0=ALU.is_ge)
                    nc.vector.tensor_scalar(ge[:, NTC - nh:],
                                            cnt[:, NTC - nh:],
                                            float(2 * cfg.K - D), None,
                                            op0=ALU.is_ge)
                else:
                    nc.vector.tensor_scalar(ge[:], cnt[:], float(cfg.K), None,
                                            op0=ALU.is_ge)
                nc.vector.tensor_tensor(dif[:], mid[:], lo[:], op=ALU.subtract)
                nc.vector.tensor_tensor(dif[:], ge[:], dif[:], op=ALU.mult)
                nc.vector.tensor_tensor(lo[:], lo[:], dif[:], op=ALU.add)
                nc.vector.tensor_tensor(dif[:], hi[:], mid[:], op=ALU.subtract)
                nc.vector.tensor_tensor(dif[:], ge[:], dif[:], op=ALU.mult)
                nc.vector.tensor_tensor(hi[:], mid[:], dif[:], op=ALU.add)
            # quantize: x8 = round(a * s8) * (|a| >= lo), s8 = 127/m8
            nc.vector.reciprocal(s8[:], m8[:])
            nc.vector.tensor_scalar(s8[:], s8[:], 127.0, None, op0=ALU.mult)
            for j in range(NTC):
                tmp = cpw.tile([128, D], F32, tag="c_tmp")
                nc.vector.tensor_scalar(tmp[:], at[j][:], s8[:, j:j + 1],
                                        MAGIC, op0=ALU.mult, op1=ALU.add)
                nc.vector.tensor_scalar(tmp[:], tmp[:], MAGIC, None,
                                        op0=ALU.subtract)
                msk = cpw.tile([128, D], F32, tag="c_msk")
                nc.vector.tensor_scalar(msk[:], absa[j][:], lo[:, j:j + 1],
                                        None, op0=ALU.is_ge)
                nc.vector.tensor_tensor(x8[j][:], tmp[:], msk[:], op=ALU.mult)
        # --- C2: transpose x8, ternarize woT, matmul, scale, store ---
        with tc.tile_pool(name="pc2", bufs=1) as cp2, \
             tc.tile_pool(name="pc2w", bufs=3) as cw2, \
             tc.tile_pool(name="pc2_ps", bufs=3, space="PSUM") as cps:
            x8T = []
            for dt in range(NDT):
                pst = cps.tile([128, TPC], BF16, tag="c_pstr")
                for j in range(NTC):
                    nc.tensor.transpose(pst[:, j * 128:(j + 1) * 128],
                                        x8[j][:, dt * 128:(dt + 1) * 128],
                                        idb[:])
                t = cp2.tile([128, TPC], BF16, tag=f"x8T_{dt}",
                             name=f"x8T_{dt}")
                nc.scalar.copy(t[:], pst[:])
                x8T.append(t)
            wot = []
            for dt in range(NDT):
                t2 = cw2.tile([128, D], F32, tag="c_wo_t")
                nc.sync.dma_start(t2[:], woT_d[dt * 128:(dt + 1) * 128, :])
                nc.vector.tensor_scalar(t2[:], t2[:], sob[:, 1:2],
                                        MAGIC, op0=ALU.mult, op1=ALU.add)
                nc.vector.tensor_scalar(t2[:], t2[:], MAGIC, -1.0,
                                        op0=ALU.subtract, op1=ALU.max)
                tb = cp2.tile([128, D], BF16, tag=f"wot_{dt}",
                              name=f"wot_{dt}")
                nc.vector.tensor_scalar(tb[:], t2[:], 1.0, None, op0=ALU.min)
                wot.append(tb)
            # y = (x8 @ wot.T) * s_wo * m8 / 127
            ysc = cp2.tile([128, NTC], F32)
            nc.vector.tensor_scalar(ysc[:], m8[:], sob[:, 0:1], None,
                                    op0=ALU.mult)
            nc.vector.tensor_scalar(ysc[:], ysc[:], 1.0 / 127.0, None,
                                    op0=ALU.mult)
            for j in range(NTC):
                ysb = cw2.tile([128, D], F32, tag="c_y")
                for fc in range(NFC):
                    ps = cps.tile([128, 512], F32, tag="c_psy")
                    for dt in range(NDT):
                        nc.tensor.matmul(ps[:],
                                         x8T[dt][:, j * 128:(j + 1) * 128],
                                         wot[dt][:, fc * 512:(fc + 1) * 512],
                                         start=(dt == 0), stop=(dt == NDT - 1))
                    nc.vector.tensor_scalar(ysb[:, fc * 512:(fc + 1) * 512],
                                            ps[:], ysc[:, j:j + 1], None,
                                            op0=ALU.mult)
                nc.sync.dma_start(y_d[j * 128:(j + 1) * 128, :], ysb[:])


# ---------------------------------------------------------------------------
# Host-side driver
# ---------------------------------------------------------------------------
_CACHED = {}


def _get_nc(cfg):
    key = (cfg.B, cfg.T, cfg.D, cfg.H, cfg.HD, cfg.chunk, cfg.qchunk,
           cfg.search_iters, cfg.no_collectives, cfg.stop_after)
    if key not in _CACHED:
        _CACHED[key] = build(cfg)
    return _CACHED[key]


def run(cfg, x, wq, wk, wv, wo, **kw):
    NT, D, FS = cfg.NT, cfg.D, cfg.FS
    x2 = np.ascontiguousarray(np.asarray(x, np.float32).reshape(NT, D))
    cosT, sinpm = rope_tables(cfg)
    idf = np.eye(128, dtype=np.float32)
    idb = idf.astype(ml_dtypes.bfloat16)
    woT = np.ascontiguousarray(np.asarray(wo, np.float32).T)
    in_maps = []
    for c in range(NCORES):
        fsl = slice(c * FS, (c + 1) * FS)
        in_maps.append({
            "x": x2,
            "wqT": np.ascontiguousarray(np.asarray(wq, np.float32).T[:, fsl]),
            "wkT": np.ascontiguousarray(np.asarray(wk, np.float32).T[:, fsl]),
            "wvT": np.ascontiguousarray(np.asarray(wv, np.float32).T[:, fsl]),
            "woT": woT,
            "cosT": cosT,
            "sinpmT": sinpm,
            "idf": idf,
            "idb": idb,
        })
    nc = _get_nc(cfg)
    res = run_bass_kernel_spmd(nc, in_maps, list(range(NCORES)), **kw)
    y = np.concatenate([res.results[c]["y"] for c in range(NCORES)], 0)
    return y.reshape(cfg.B, cfg.T, cfg.D)


def kernel(x, wq, wk, wv, wo):
    return run(Cfg(), x, wq, wk, wv, wo)


if __name__ == "__main__":
    cfg = Cfg()
    rng = np.random.default_rng(0)
    x = rng.standard_normal((cfg.B, cfg.T, cfg.D)).astype(np.float32)
    ws = [(rng.standard_normal((cfg.D, cfg.D)) * 0.02).astype(np.float32)
          for _ in range(4)]
    y = kernel(x, *ws)
    print("out", y.shape, y.dtype, float(np.abs(y).max()))



# revision 2
# speedup vs baseline: 45.1973x; 45.1973x over previous
"""BitAttention (ternary-weight attention with int4/topk-int8 activation quant)
on 8 Trainium2 NeuronCores.

Sharding: tensor-parallel over heads for qkv-proj + SDPA (heads/8 per core),
AllToAll re-shard to token-parallel for the topk+int8 o-projection.

Numerics: quantized values are exact small integers, so qkv/o projections run
as exact integer arithmetic in bf16 matmuls (fp32 PSUM accumulate). Attention
(rope'd q/k real-valued) runs in fp32 matmuls. Softmax exp on ACT. Top-k
per-row threshold found by binary search on the |value| axis; per-token scales
folded into rope tables / exp bias / output scaling.
"""
import math
import numpy as np
import ml_dtypes

# ---------------------------------------------------------------------------
# TileContext patches for this walrus build (single sem-wait per instruction).
# ---------------------------------------------------------------------------
import re as _re
import concourse.mybir as mybir
import concourse.bass as bass
import concourse.tile as tile
from concourse.tile import TileContext, ScopedClock, VectorClock
from concourse.bass_utils import run_bass_kernel_spmd

_carrier_seq = [0]
_orig_add_instruction = TileContext._add_instruction


def _patched_add_instruction(self, inst):
    si = inst.sync_info
    if si is not None and si.on_wait is not None and len(si.on_wait) > 1:
        waits = list(si.on_wait)
        for w in waits[:-1]:
            _carrier_seq[0] += 1
            carrier = mybir.InstEventSemaphore(
                name=f"waitc_{_carrier_seq[0]}_{inst.name}",
                engine=inst.engine,
                ins=[],
                outs=[],
                sync_info=mybir.SyncInfo(on_wait=[w], on_update=[]),
            )
            _orig_add_instruction(self, carrier)
        si.on_wait = [waits[-1]]
        inst.sync_info = si
    _orig_add_instruction(self, inst)


def _clock_ticks(clock):
    m = _re.match(r"VectorClock\((\[.*\])\)", repr(clock))
    return eval(m.group(1))


def _patched_drain_and_barrier(self, tick_clock, wait_clock):
    nc = self.nc
    ticks = _clock_ticks(tick_clock.global_clock)
    n = len(ticks)
    for i, t in enumerate(ticks):
        if t > 0:
            d = nc.sync.drain()
            vci = VectorClock([t if j == i else 0 for j in range(n)])
            wait_clock.add_sem_waits(d.ins, ScopedClock({None: vci}))
    nc.sync.drain()
    nc.all_engine_barrier()
    assert self.sems is not None
    popped = nc._tile_sem_poison_stack.pop()
    assert popped is self._sem_poison
    nc.clear_and_free_semaphores(list(self.sems.allocated().values()))
    nc.all_engine_barrier()


TileContext._add_instruction = _patched_add_instruction
TileContext._drain_and_barrier = _patched_drain_and_barrier

# ---------------------------------------------------------------------------

F32 = mybir.dt.float32
BF16 = mybir.dt.bfloat16
AF = mybir.ActivationFunctionType
ALU = mybir.AluOpType
AX = mybir.AxisListType
MAGIC = 1.5 * 2.0 ** 23
EPS = 1e-5
THETA = 10000.0
TOPK_RATIO = 0.55
NCORES = 8


class Cfg:
    def __init__(self, B=2, T=2048, D=2048, H=16, HD=128, chunk=256, qchunk=256,
                 search_iters=26, no_collectives=False, stop_after=''):
        self.B, self.T, self.D, self.H, self.HD = B, T, D, H, HD
        self.NT = B * T
        self.HPC = H // NCORES            # heads per core
        self.FS = self.HPC * HD           # feature slice per core
        self.chunk = chunk                # phase-A token chunk
        self.qchunk = qchunk              # attention q chunk
        self.TPC = self.NT // NCORES      # tokens per core in phase C
        self.K = max(1, int(TOPK_RATIO * D))
        self.search_iters = search_iters
        self.no_collectives = no_collectives
        self.stop_after = stop_after
        assert self.NT % 128 == 0 and D % 512 == 0 and HD % 2 == 0
        assert T % qchunk == 0 and self.NT % chunk == 0 and chunk % 128 == 0
        assert self.TPC % 128 == 0 and HD <= 128 and self.FS % 128 == 0
        assert D == H * HD


def rope_tables(cfg):
    hd, T = cfg.HD, cfg.T
    inv = 1.0 / THETA ** (np.arange(0, hd, 2, dtype=np.float32) / hd)
    freqs = np.arange(T, dtype=np.float32)[:, None] * inv[None, :]
    emb = np.concatenate([freqs, freqs], axis=1)          # (T, hd)
    cos = np.cos(emb).astype(np.float32)
    sin = np.sin(emb).astype(np.float32)
    cosT = np.concatenate([cos] * cfg.B, 0).T.copy()      # (hd, NT)
    sinT = np.concatenate([sin] * cfg.B, 0).T.copy()
    sin_pm = sinT.copy()
    sin_pm[: hd // 2] = -sin_pm[: hd // 2]                # rotate-half signs
    return np.ascontiguousarray(cosT), np.ascontiguousarray(sin_pm)


def build(cfg: Cfg):
    nc = bass.Bass("TRN2", target_bir_lowering=False, debug=False,
                   num_devices=NCORES)
    NT, D, HD, FS, TPC = cfg.NT, cfg.D, cfg.HD, cfg.FS, cfg.TPC

    x_d = nc.dram_tensor("x", [NT, D], F32, kind="ExternalInput")
    wqT_d = nc.dram_tensor("wqT", [D, FS], F32, kind="ExternalInput")
    wkT_d = nc.dram_tensor("wkT", [D, FS], F32, kind="ExternalInput")
    wvT_d = nc.dram_tensor("wvT", [D, FS], F32, kind="ExternalInput")
    woT_d = nc.dram_tensor("woT", [D, D], F32, kind="ExternalInput")
    cos_d = nc.dram_tensor("cosT", [HD, NT], F32, kind="ExternalInput")
    sin_d = nc.dram_tensor("sinpmT", [HD, NT], F32, kind="ExternalInput")
    idf_d = nc.dram_tensor("idf", [128, 128], F32, kind="ExternalInput")
    idb_d = nc.dram_tensor("idb", [128, 128], BF16, kind="ExternalInput")
    y_d = nc.dram_tensor("y", [TPC, D], F32, kind="ExternalOutput")

    with TileContext(nc, pool_alloc_mode="queue") as tc:
        _body(nc, tc, cfg, x_d, wqT_d, wkT_d, wvT_d, woT_d, cos_d, sin_d,
              idf_d, idb_d, y_d)
    return nc


def _body(nc, tc, cfg, x_d, wqT_d, wkT_d, wvT_d, woT_d, cos_d, sin_d,
          idf_d, idb_d, y_d):
    NT, D, HD, HPC, FS = cfg.NT, cfg.D, cfg.HD, cfg.HPC, cfg.FS
    NTT, NDT = NT // 128, D // 128
    CH = cfg.chunk
    NCH, CTT = NT // CH, CH // 128
    QC, KT, NQC = cfg.qchunk, cfg.T // 128, cfg.T // cfg.qchunk
    TPC, FTQ = cfg.TPC, FS // 128
    SQRT_SCALE = float(1.0 / math.sqrt(HD))
    HH = HD // 2

    with tc.tile_pool(name="persist", bufs=1) as pp, \
         tc.tile_pool(name="dram", bufs=1, space="DRAM") as dramp:
        idf = pp.tile([128, 128], F32)
        nc.sync.dma_start(idf[:], idf_d[:])
        idb = pp.tile([128, 128], BF16)
        nc.sync.dma_start(idb[:], idb_d[:])
        ones_col = pp.tile([128, 1], F32)        # partition sums (lhsT)
        nc.gpsimd.memset(ones_col[:], 1.0)
        ones_row = pp.tile([1, 128], F32)        # partition broadcast (lhsT)
        nc.gpsimd.memset(ones_row[:], 1.0)
        # per token-tile columns (tokens on partitions)
        inv_sx = pp.tile([128, NTT], F32)
        ln_sv = pp.tile([128, NTT], F32)
        rinv_sv = pp.tile([128, NTT], F32)
        ws_s = pp.tile([128, 4], F32)   # bcast weight scales s_w (q,k,v,o)
        ws_r = pp.tile([128, 4], F32)   # bcast 1/(s_w+eps)
        sob = pp.tile([128, 2], F32)    # bcast (s_wo, 1/(s_wo+eps))
        a2a_in = dramp.tile([NT, FS], F32)
        a2a_out = dramp.tile([NT, FS], F32)

        with tc.tile_pool(name="pAB", bufs=1) as pab:
            # roped q/k [FS, NT] fp32; integer v in token layout (fp32 ints)
            qT = [pab.tile([128, NT], F32, tag=f"qT{i}", name=f"qT{i}") for i in range(FTQ)]
            kT = [pab.tile([128, NT], F32, tag=f"kT{i}", name=f"kT{i}") for i in range(FTQ)]
            vtok = [pab.tile([128, FS], F32, tag=f"vtok{i}", name=f"vtok{i}") for i in range(NTT)]

            with tc.tile_pool(name="pW", bufs=1) as pw:
                # ternarized weight slices (bf16 ints), persist through phase A
                wqt = [pw.tile([128, FS], BF16, tag=f"wqt{i}", name=f"wqt{i}") for i in range(NDT)]
                wkt = [pw.tile([128, FS], BF16, tag=f"wkt{i}", name=f"wkt{i}") for i in range(NDT)]
                wvt = [pw.tile([128, FS], BF16, tag=f"wvt{i}", name=f"wvt{i}") for i in range(NDT)]
                _phase_w(nc, tc, cfg, dramp, wqT_d, wkT_d, wvT_d, woT_d,
                         ones_col, ones_row, ws_s, ws_r, wqt, wkt, wvt)
                if cfg.stop_after == 'W':
                    return
                _phase_a(nc, tc, cfg, x_d, cos_d, sin_d, idf, idb, ones_row,
                         ws_s, inv_sx, ln_sv, rinv_sv, wqt, wkt, wvt,
                         qT, kT, vtok)
            if cfg.stop_after == 'A':
                return
            wom = _WoMean(nc, tc, cfg, woT_d, ones_col, ones_row, sob)
            _phase_b(nc, tc, cfg, idf, ones_col, ones_row, ln_sv, rinv_sv,
                     qT, kT, vtok, a2a_in, wom)
            wom.finish()
        if cfg.stop_after == 'B':
            return

        if cfg.no_collectives:
            nc.sync.dma_start(a2a_out[:], a2a_in[:])
        else:
            nc.gpsimd.collective_compute(
                "AllToAll", ALU.bypass, replica_groups=[list(range(NCORES))],
                ins=[a2a_in[:].opt()], outs=[a2a_out[:].opt()])
        _phase_c(nc, tc, cfg, woT_d, idb, ws_s, ws_r, ones_col, ones_row, sob, a2a_out, y_d)


def _phase_w(nc, tc, cfg, dramp, wqT_d, wkT_d, wvT_d, woT_d, ones_col,
             ones_row, ws_s, ws_r, wqt, wkt, wvt):
    D, FS = cfg.D, cfg.FS
    NDT = D // 128
    with tc.tile_pool(name="ph_w", bufs=1) as wp, \
         tc.tile_pool(name="ph_w_ps", bufs=2, space="PSUM") as wps, \
         tc.tile_pool(name="ph_w1", bufs=1) as wp1:
        partials = wp1.tile([1, 4], F32)
        wraw = {}
        for j, wd in enumerate([wqT_d, wkT_d, wvT_d]):
            acc = wp1.tile([128, 1], F32, tag=f"wacc{j}", name=f"wacc{j}")
            nc.gpsimd.memset(acc[:], 0.0)
            for dt in range(NDT):
                t = wp.tile([128, FS], F32, tag=f"wld_{j}_{dt}",
                            name=f"wld_{j}_{dt}")
                nc.sync.dma_start(t[:], wd[dt * 128:(dt + 1) * 128, :])
                wraw[(j, dt)] = t
                r = wp1.tile([128, 1], F32, tag="wred")
                nc.vector.tensor_reduce(r[:], t[:], axis=AX.X, op=ALU.add,
                                        apply_absolute_value=True)
                nc.vector.tensor_tensor(acc[:], acc[:], r[:], op=ALU.add)
            ps = wps.tile([1, 1], F32, tag="w_ps1")
            nc.tensor.matmul(ps[:], acc[:], ones_col[:, 0:1], start=True,
                             stop=True)
            nc.vector.tensor_copy(partials[:, j:j + 1], ps[:])
        nc.gpsimd.memset(partials[:, 3:4], 0.0)
        ar_in = dramp.tile([1, 4], F32)
        ar_out = dramp.tile([1, 4], F32, addr_space="Shared")
        nc.sync.dma_start(ar_in[:], partials[:])
        if cfg.no_collectives:
            nc.sync.dma_start(ar_out[:], ar_in[:])
        else:
            nc.gpsimd.collective_compute(
                "AllReduce", ALU.add, replica_groups=[list(range(NCORES))],
                ins=[ar_in[:].opt()], outs=[ar_out[:].opt()])
        sums = wp1.tile([1, 4], F32)
        nc.sync.dma_start(sums[:], ar_out[:])
        s_row = wp1.tile([1, 4], F32)
        nc.vector.tensor_scalar(s_row[:], sums[:], 1.0 / (float(D) * float(D)),
                                None, op0=ALU.mult)
        r_row = wp1.tile([1, 4], F32)
        nc.vector.tensor_scalar(r_row[:], s_row[:], EPS, None, op0=ALU.add)
        nc.vector.reciprocal(r_row[:], r_row[:])
        ps_b = wps.tile([128, 4], F32, tag="w_psb")
        nc.tensor.matmul(ps_b[:], ones_row[:], s_row[:], start=True, stop=True)
        nc.scalar.copy(ws_s[:], ps_b[:])
        ps_b2 = wps.tile([128, 4], F32, tag="w_psb")
        nc.tensor.matmul(ps_b2[:], ones_row[:], r_row[:], start=True,
                         stop=True)
        nc.scalar.copy(ws_r[:], ps_b2[:])
        # ternarize in place from the resident raw tiles
        for j, dst in enumerate([wqt, wkt, wvt]):
            for dt in range(NDT):
                t = wraw[(j, dt)]
                nc.vector.tensor_scalar(t[:], t[:], ws_r[:, j:j + 1], MAGIC,
                                        op0=ALU.mult, op1=ALU.add)
                nc.vector.tensor_scalar(t[:], t[:], MAGIC, -1.0,
                                        op0=ALU.subtract, op1=ALU.max)
                nc.vector.tensor_scalar(dst[dt][:], t[:], 1.0, None,
                                        op0=ALU.min)


def _phase_a(nc, tc, cfg, x_d, cos_d, sin_d, idf, idb, ones_row, ws_s,
             inv_sx, ln_sv, rinv_sv, wqt, wkt, wvt, qT, kT, vtok):
    D, HD, FS = cfg.D, cfg.HD, cfg.FS
    NDT = D // 128
    CH = cfg.chunk
    NCH, CTT = cfg.NT // CH, CH // 128
    FTQ = FS // 128
    HH = HD // 2
    with tc.tile_pool(name="ph_a", bufs=2) as ap, \
         tc.tile_pool(name="ph_a3", bufs=3) as ap3, \
         tc.tile_pool(name="ph_a_ps", bufs=3, space="PSUM") as aps, \
         tc.tile_pool(name="ph_a_ps3", bufs=4, space="PSUM") as aps3:
        for ch in range(NCH):
            t0 = ch * CH
            xq = [None] * CTT
            for j in range(CTT):
                tt = t0 // 128 + j
                xt = ap.tile([128, D], F32, tag="xload")
                nc.sync.dma_start(xt[:], x_d[tt * 128:(tt + 1) * 128, :])
                m = ap.tile([128, 1], F32, tag="xm")
                nc.vector.tensor_reduce(m[:], xt[:], axis=AX.X, op=ALU.max,
                                        apply_absolute_value=True)
                nc.vector.tensor_scalar(m[:], m[:], EPS, None, op0=ALU.max)
                nc.vector.tensor_scalar(inv_sx[:, tt:tt + 1], m[:], 1.0 / 7.0,
                                        None, op0=ALU.mult)
                sx = ap.tile([128, 1], F32, tag="xs")
                nc.vector.reciprocal(sx[:], m[:])
                nc.vector.tensor_scalar(sx[:], sx[:], 7.0, None, op0=ALU.mult)
                sv = ap.tile([128, 1], F32, tag="xsv")
                nc.vector.tensor_tensor(sv[:], inv_sx[:, tt:tt + 1],
                                        ws_s[:, 2:3], op=ALU.mult)
                nc.scalar.activation(ln_sv[:, tt:tt + 1], sv[:], AF.Ln)
                nc.vector.reciprocal(rinv_sv[:, tt:tt + 1], sv[:])
                xqj = ap.tile([128, D], BF16, tag="xq")
                tmp = ap.tile([128, D], F32, tag="xtmp")
                nc.vector.tensor_scalar(tmp[:], xt[:], sx[:], MAGIC,
                                        op0=ALU.mult, op1=ALU.add)
                nc.vector.tensor_scalar(xqj[:], tmp[:], MAGIC, None,
                                        op0=ALU.subtract)
                xq[j] = xqj
            # transpose xq -> xqT tiles [128 d, CH] (bf16 ints)
            xqT = [None] * NDT
            for dt in range(NDT):
                pst = aps.tile([128, CH], BF16, tag="ps_misc")
                for j in range(CTT):
                    nc.tensor.transpose(pst[:, j * 128:(j + 1) * 128],
                                        xq[j][:, dt * 128:(dt + 1) * 128],
                                        idb[:])
                xqT[dt] = ap.tile([128, CH], BF16, tag=f"xqT{dt}", name=f"xqT{dt}")
                nc.scalar.copy(xqT[dt][:], pst[:])
            # scaled rope tables for this chunk
            cos_c = ap.tile([HD, CH], F32, tag="cos_c")
            nc.sync.dma_start(cos_c[:], cos_d[:, t0:t0 + CH])
            sin_c = ap.tile([HD, CH], F32, tag="sin_c")
            nc.sync.dma_start(sin_c[:], sin_d[:, t0:t0 + CH])
            tabs = {}
            for (wj, nm) in ((0, 'q'), (1, 'k')):
                colp = ap.tile([128, CTT], F32, tag="colp")
                nc.vector.tensor_scalar(colp[:],
                                        inv_sx[:, t0 // 128:t0 // 128 + CTT],
                                        ws_s[:, wj:wj + 1], None, op0=ALU.mult)
                pst = aps.tile([128, 128], F32, tag="ps_misc")
                nc.tensor.transpose(pst[:CTT, :], colp[:], idf[:])
                srow_t = ap.tile([CTT, 128], F32, tag="srowt")
                nc.scalar.copy(srow_t[:], pst[:CTT, :])
                srow = ap.tile([1, CH], F32, tag="srow")
                nc.sync.dma_start(srow[:], srow_t[:])
                psb = aps.tile([HD, CH], F32, tag="ps_misc")
                nc.tensor.matmul(psb[:], ones_row[:, :HD], srow[:],
                                 start=True, stop=True)
                sb = ap.tile([HD, CH], F32, tag=f"sb_{nm}", name=f"sb_{nm}")
                nc.scalar.copy(sb[:], psb[:])
                tc_t = ap.tile([HD, CH], F32, tag=f"tc_{nm}", name=f"tc_{nm}")
                nc.vector.tensor_tensor(tc_t[:], cos_c[:], sb[:], op=ALU.mult)
                ts_t = ap.tile([HD, CH], F32, tag=f"ts_{nm}", name=f"ts_{nm}")
                nc.vector.tensor_tensor(ts_t[:], sin_c[:], sb[:], op=ALU.mult)
                tabs[nm] = (tc_t, ts_t)
            # projections + rope drains
            for wt, nm, dstT in ((wqt, 'q', qT), (wkt, 'k', kT)):
                table_c, table_s = tabs[nm]
                for ft in range(FTQ):
                    ps = aps3.tile([128, CH], F32, tag="ps_proj")
                    for dt in range(NDT):
                        nc.tensor.matmul(ps[:],
                                         wt[dt][:, ft * 128:(ft + 1) * 128],
                                         xqT[dt][:], start=(dt == 0),
                                         stop=(dt == NDT - 1))
                    dtile = dstT[ft]
                    for hb in range(128 // HD):
                        fo = hb * HD
                        t1 = ap3.tile([HD, CH], F32, tag="ropet1")
                        nc.vector.tensor_tensor(t1[:], ps[fo:fo + HD, :],
                                                table_c[:], op=ALU.mult)
                        t2 = ap3.tile([HD, CH], F32, tag="ropet2")
                        nc.vector.tensor_tensor(t2[:HH, :],
                                                ps[fo + HH:fo + HD, :],
                                                table_s[:HH, :], op=ALU.mult)
                        nc.vector.tensor_tensor(t2[HH:, :], ps[fo:fo + HH, :],
                                                table_s[HH:, :], op=ALU.mult)
                        nc.vector.tensor_tensor(dtile[fo:fo + HD, t0:t0 + CH],
                                                t1[:], t2[:], op=ALU.add)
            # v: integer result, drain fp32, transpose to token layout
            for ft in range(FTQ):
                ps = aps3.tile([128, CH], F32, tag="ps_proj")
                for dt in range(NDT):
                    nc.tensor.matmul(ps[:], wvt[dt][:, ft * 128:(ft + 1) * 128],
                                     xqT[dt][:], start=(dt == 0),
                                     stop=(dt == NDT - 1))
                vtmp = ap3.tile([128, CH], F32, tag="vtmp")
                nc.scalar.copy(vtmp[:], ps[:])
                for j in range(CTT):
                    tt = t0 // 128 + j
                    pst = aps.tile([128, 128], F32, tag="ps_misc")
                    nc.tensor.transpose(pst[:], vtmp[:, j * 128:(j + 1) * 128],
                                        idf[:])
                    nc.scalar.copy(vtok[tt][:, ft * 128:(ft + 1) * 128],
                                   pst[:])


class _WoMean:
    """mean|wo| pass, emitted one tile per attention iteration so the DVE/DMA
    work interleaves with phase B instead of serializing before it."""

    def __init__(self, nc, tc, cfg, woT_d, ones_col, ones_row, sob):
        self.nc, self.tc, self.cfg = nc, tc, cfg
        self.woT_d, self.ones_col, self.ones_row, self.sob = (
            woT_d, ones_col, ones_row, sob)
        self.NDT = cfg.D // 128
        self._cms = [tc.tile_pool(name="pwo", bufs=2),
                     tc.tile_pool(name="pwo1", bufs=1),
                     tc.tile_pool(name="pwo_ps", bufs=1, space="PSUM")]
        self.wp = self._cms[0].__enter__()
        self.wp1 = self._cms[1].__enter__()
        self.wps = self._cms[2].__enter__()
        self.acc = self.wp1.tile([128, 1], F32, name="wo_acc")
        nc.gpsimd.memset(self.acc[:], 0.0)
        self.done = 0

    def step(self):
        if self.done >= self.NDT:
            return
        nc, D = self.nc, self.cfg.D
        dt = self.done
        self.done += 1
        t = self.wp.tile([128, D], F32, tag="wo_ld", name="wo_ld")
        nc.sync.dma_start(t[:], self.woT_d[dt * 128:(dt + 1) * 128, :])
        r = self.wp1.tile([128, 1], F32, tag="wo_red", name="wo_red")
        nc.vector.tensor_reduce(r[:], t[:], axis=AX.X, op=ALU.add,
                                apply_absolute_value=True)
        nc.vector.tensor_tensor(self.acc[:], self.acc[:], r[:], op=ALU.add)

    def finish(self):
        while self.done < self.NDT:
            self.step()
        nc, D = self.nc, self.cfg.D
        pss = self.wps.tile([1, 1], F32, tag="wo_ps", name="wo_ps")
        nc.tensor.matmul(pss[:], self.acc[:], self.ones_col[:, 0:1],
                         start=True, stop=True)
        so_s = self.wp1.tile([1, 1], F32, name="so_s")
        nc.vector.tensor_scalar(so_s[:], pss[:], 1.0 / (float(D) * float(D)),
                                None, op0=ALU.mult)
        so_r = self.wp1.tile([1, 1], F32, name="so_r")
        nc.vector.tensor_scalar(so_r[:], so_s[:], EPS, None, op0=ALU.add)
        nc.vector.reciprocal(so_r[:], so_r[:])
        sr2 = self.wp1.tile([1, 2], F32, name="sr2")
        nc.vector.tensor_copy(sr2[:, 0:1], so_s[:])
        nc.vector.tensor_copy(sr2[:, 1:2], so_r[:])
        psb = self.wps.tile([128, 2], F32, tag="wo_ps", name="wo_psb")
        nc.tensor.matmul(psb[:], self.ones_row[:], sr2[:], start=True,
                         stop=True)
        nc.scalar.copy(self.sob[:], psb[:])
        for cm in reversed(self._cms):
            cm.__exit__(None, None, None)


def _phase_b(nc, tc, cfg, idf, ones_col, ones_row, ln_sv, rinv_sv,
             qT, kT, vtok, a2a_in, wom=None):
    HD, HPC = cfg.HD, cfg.HPC
    QC, KT, NQC = cfg.qchunk, cfg.T // 128, cfg.T // cfg.qchunk
    with tc.tile_pool(name="ph_b_p", bufs=2) as bp, \
         tc.tile_pool(name="ph_b_pt", bufs=2) as bpt, \
         tc.tile_pool(name="ph_b_ps_sc", bufs=3, space="PSUM") as ps_sc, \
         tc.tile_pool(name="ph_b_ps_o", bufs=2, space="PSUM") as ps_o, \
         tc.tile_pool(name="ph_b_ps_m", bufs=2, space="PSUM") as ps_m:
        for b in range(cfg.B):
            for hh in range(HPC):
                fo = hh * HD
                ftile, fin = fo // 128, fo % 128
                for qc in range(NQC):
                    q0 = b * cfg.T + qc * QC
                    pT = [bpt.tile([128, QC], F32, tag=f"pT{i}", name=f"pT{i}")
                          for i in range(KT)]
                    den = bp.tile([128, QC], F32, tag="den")
                    outp = ps_o.tile([HD, QC], F32, tag="outp")
                    if wom is not None:
                        wom.step()
                    for kt in range(KT):
                        ktt = (b * cfg.T) // 128 + kt
                        k0 = b * cfg.T + kt * 128
                        ssc = ps_sc.tile([128, QC], F32, tag="ssc")
                        nc.tensor.matmul(
                            ssc[:], kT[ftile][fin:fin + HD, k0:k0 + 128],
                            qT[ftile][fin:fin + HD, q0:q0 + QC],
                            start=True, stop=True)
                        nc.scalar.activation(pT[kt][:], ssc[:], AF.Exp,
                                             bias=ln_sv[:, ktt:ktt + 1],
                                             scale=SQRT_SCALE_OF(cfg))
                        if kt == 0:
                            nc.vector.tensor_scalar(den[:], pT[kt][:],
                                                    rinv_sv[:, ktt:ktt + 1],
                                                    None, op0=ALU.mult)
                        else:
                            nc.vector.scalar_tensor_tensor(
                                den[:], in0=pT[kt][:],
                                scalar=rinv_sv[:, ktt:ktt + 1], in1=den[:],
                                op0=ALU.mult, op1=ALU.add)
                        nc.tensor.matmul(outp[:], vtok[ktt][:, fo:fo + HD],
                                         pT[kt][:], start=(kt == 0),
                                         stop=(kt == KT - 1))
                    dps = ps_m.tile([1, QC], F32, tag="ps_misc")
                    nc.tensor.matmul(dps[:], ones_col[:], den[:], start=True,
                                     stop=True)
                    drow = bp.tile([1, QC], F32, tag="drow")
                    nc.vector.reciprocal(drow[:], dps[:])
                    rdb = ps_m.tile([HD, QC], F32, tag="ps_misc")
                    nc.tensor.matmul(rdb[:], ones_row[:, :HD], drow[:],
                                     start=True, stop=True)
                    osb = bp.tile([HD, QC], F32, tag="osb")
                    nc.scalar.copy(osb[:], outp[:])
                    nc.vector.tensor_tensor(osb[:], osb[:], rdb[:],
                                            op=ALU.mult)
                    for j in range(QC // 128):
                        pst = ps_m.tile([128, HD], F32, tag="ps_misc")
                        nc.tensor.transpose(pst[:],
                                            osb[:, j * 128:(j + 1) * 128],
                                            idf[:])
                        stg = bp.tile([128, HD], F32, tag="stg")
                        nc.scalar.copy(stg[:], pst[:])
                        r0 = q0 + j * 128
                        nc.sync.dma_start(a2a_in[r0:r0 + 128, fo:fo + HD],
                                          stg[:])


def SQRT_SCALE_OF(cfg):
    return float(1.0 / math.sqrt(cfg.HD))


def _phase_c(nc, tc, cfg, woT_d, idb, ws_s, ws_r, ones_col, ones_row, sob, a2a_out, y_d):
    D, TPC = cfg.D, cfg.TPC
    NDT = D // 128
    NTC = TPC // 128
    NFC = D // 512
    with tc.tile_pool(name="pc0", bufs=1) as pc0:
        m8 = pc0.tile([128, NTC], F32)
        lo = pc0.tile([128, NTC], F32)
        s8 = pc0.tile([128, NTC], F32)
        x8 = [pc0.tile([128, D], BF16, tag=f"x8_{j}", name=f"x8_{j}")
              for j in range(NTC)]
        # --- C1: load, abs, threshold search, int8 quant + mask ---
        with tc.tile_pool(name="pc1", bufs=1) as cp1, \
             tc.tile_pool(name="pc1w", bufs=3) as cpw:
            a2a_v = a2a_out[:].rearrange("(s t) f -> t s f", s=NCORES)
            at, absa = [], []
            for j in range(NTC):
                t = cp1.tile([128, D], F32, tag=f"at{j}", name=f"at{j}")
                nc.sync.dma_start(t[:].rearrange("p (s f) -> p s f", s=NCORES),
                                  a2a_v[j * 128:(j + 1) * 128])
                at.append(t)
                ab = cp1.tile([128, D], F32, tag=f"ab{j}", name=f"ab{j}")
                nc.scalar.activation(ab[:], t[:], AF.Abs)
                absa.append(ab)
                nc.vector.tensor_reduce(m8[:, j:j + 1], ab[:], axis=AX.X,
                                        op=ALU.max)
            nc.vector.tensor_scalar(m8[:], m8[:], EPS, None, op0=ALU.max)
            # binary search for the k-th largest |a| per row
            nc.gpsimd.memset(lo[:], 0.0)
            hi = cp1.tile([128, NTC], F32)
            nc.vector.tensor_scalar(hi[:], m8[:], 1.0001, None, op0=ALU.mult)
            mid = cp1.tile([128, NTC], F32)
            nmid = cp1.tile([128, NTC], F32)
            cnt = cp1.tile([128, NTC], F32)
            ge = cp1.tile([128, NTC], F32)
            dif = cp1.tile([128, NTC], F32)
            junk = cp1.tile([128, D], F32)
            junka = cp1.tile([128, D], F32)
            # first iters: upper half of the token tiles counted on ACT via
            # Sign+accum (acc = #above - #below); later iters all on DVE
            # (exact >= semantics near convergence).
            nh = NTC // 2
            act_iters = max(0, cfg.search_iters - 10) if nh else 0
            for it in range(cfg.search_iters):
                nc.vector.tensor_tensor(mid[:], lo[:], hi[:], op=ALU.add)
                nc.vector.tensor_scalar(mid[:], mid[:], 0.5, None, op0=ALU.mult)
                use_act = it < act_iters
                if use_act:
                    nc.vector.tensor_scalar(nmid[:], mid[:], -1.0, None,
                                            op0=ALU.mult)
                for j in range(NTC):
                    if use_act and j >= NTC - nh:
                        nc.scalar.activation(junka[:], absa[j][:], AF.Sign,
                                             bias=nmid[:, j:j + 1],
                                             accum_out=cnt[:, j:j + 1])
                    else:
                        nc.vector.tensor_scalar(junk[:], absa[j][:],
                                                mid[:, j:j + 1], None,
                                                op0=ALU.is_ge, op1=ALU.add,
                                                accum_out=cnt[:, j:j + 1])
                if use_act:
                    nc.vector.tensor_scalar(ge[:, :NTC - nh],
                                            cnt[:, :NTC - nh], float(cfg.K),
                                            None, op0=ALU.is_ge)
                    nc.vector.tensor_scalar(ge[:, NTC - nh:],
                                            cnt[:, NTC - nh:],
                                            float(2 * cfg.K - D), None,
                                            op0=ALU.is_ge)
                else:
                    nc.vector.tensor_scalar(ge[:], cnt[:], float(cfg.K), None,
                                            op0=ALU.is_ge)
                nc.vector.tensor_tensor(dif[:], mid[:], lo[:], op=ALU.subtract)
                nc.vector.tensor_tensor(dif[:], ge[:], dif[:], op=ALU.mult)
                nc.vector.tensor_tensor(lo[:], lo[:], dif[:], op=ALU.add)
                nc.vector.tensor_tensor(dif[:], hi[:], mid[:], op=ALU.subtract)
                nc.vector.tensor_tensor(dif[:], ge[:], dif[:], op=ALU.mult)
                nc.vector.tensor_tensor(hi[:], mid[:], dif[:], op=ALU.add)
            # quantize: x8 = round(a * s8) * (|a| >= lo), s8 = 127/m8
            nc.vector.reciprocal(s8[:], m8[:])
            nc.vector.tensor_scalar(s8[:], s8[:], 127.0, None, op0=ALU.mult)
            for j in range(NTC):
                tmp = cpw.tile([128, D], F32, tag="c_tmp")
                nc.vector.tensor_scalar(tmp[:], at[j][:], s8[:, j:j + 1],
                                        MAGIC, op0=ALU.mult, op1=ALU.add)
                nc.vector.tensor_scalar(tmp[:], tmp[:], MAGIC, None,
                                        op0=ALU.subtract)
                msk = cpw.tile([128, D], F32, tag="c_msk")
                nc.vector.tensor_scalar(msk[:], absa[j][:], lo[:, j:j + 1],
                                        None, op0=ALU.is_ge)
                nc.vector.tensor_tensor(x8[j][:], tmp[:], msk[:], op=ALU.mult)
        # --- C2: transpose x8, ternarize woT, matmul, scale, store ---
        with tc.tile_pool(name="pc2", bufs=1) as cp2, \
             tc.tile_pool(name="pc2w", bufs=3) as cw2, \
             tc.tile_pool(name="pc2_ps", bufs=3, space="PSUM") as cps:
            x8T = []
            for dt in range(NDT):
                pst = cps.tile([128, TPC], BF16, tag="c_pstr")
                for j in range(NTC):
                    nc.tensor.transpose(pst[:, j * 128:(j + 1) * 128],
                                        x8[j][:, dt * 128:(dt + 1) * 128],
                                        idb[:])
                t = cp2.tile([128, TPC], BF16, tag=f"x8T_{dt}",
                             name=f"x8T_{dt}")
                nc.scalar.copy(t[:], pst[:])
                x8T.append(t)
            wot = []
            for dt in range(NDT):
                t2 = cw2.tile([128, D], F32, tag="c_wo_t")
                nc.sync.dma_start(t2[:], woT_d[dt * 128:(dt + 1) * 128, :])
                nc.vector.tensor_scalar(t2[:], t2[:], sob[:, 1:2],
                                        MAGIC, op0=ALU.mult, op1=ALU.add)
                nc.vector.tensor_scalar(t2[:], t2[:], MAGIC, -1.0,
                                        op0=ALU.subtract, op1=ALU.max)
                tb = cp2.tile([128, D], BF16, tag=f"wot_{dt}",
                              name=f"wot_{dt}")
                nc.vector.tensor_scalar(tb[:], t2[:], 1.0, None, op0=ALU.min)
                wot.append(tb)
            # y = (x8 @ wot.T) * s_wo * m8 / 127
            ysc = cp2.tile([128, NTC], F32)
            nc.vector.tensor_scalar(ysc[:], m8[:], sob[:, 0:1], None,
                                    op0=ALU.mult)
            nc.vector.tensor_scalar(ysc[:], ysc[:], 1.0 / 127.0, None,
                                    op0=ALU.mult)
            for j in range(NTC):
                ysb = cw2.tile([128, D], F32, tag="c_y")
                for fc in range(NFC):
                    ps = cps.tile([128, 512], F32, tag="c_psy")
                    for dt in range(NDT):
                        nc.tensor.matmul(ps[:],
                                         x8T[dt][:, j * 128:(j + 1) * 128],
                                         wot[dt][:, fc * 512:(fc + 1) * 512],
                                         start=(dt == 0), stop=(dt == NDT - 1))
                    nc.vector.tensor_scalar(ysb[:, fc * 512:(fc + 1) * 512],
                                            ps[:], ysc[:, j:j + 1], None,
                                            op0=ALU.mult)
                nc.sync.dma_start(y_d[j * 128:(j + 1) * 128, :], ysb[:])


# ---------------------------------------------------------------------------
# Host-side driver
# ---------------------------------------------------------------------------
_CACHED = {}


def _get_nc(cfg):
    key = (cfg.B, cfg.T, cfg.D, cfg.H, cfg.HD, cfg.chunk, cfg.qchunk,
           cfg.search_iters, cfg.no_collectives, cfg.stop_after)
    if key not in _CACHED:
        _CACHED[key] = build(cfg)
    return _CACHED[key]


def make_in_maps(cfg, x, wq, wk, wv, wo):
    NT, D, FS = cfg.NT, cfg.D, cfg.FS
    x2 = np.ascontiguousarray(np.asarray(x, np.float32).reshape(NT, D))
    cosT, sinpm = rope_tables(cfg)
    idf = np.eye(128, dtype=np.float32)
    idb = idf.astype(ml_dtypes.bfloat16)
    woT = np.ascontiguousarray(np.asarray(wo, np.float32).T)
    in_maps = []
    for c in range(NCORES):
        fsl = slice(c * FS, (c + 1) * FS)
        in_maps.append({
            "x": x2,
            "wqT": np.ascontiguousarray(np.asarray(wq, np.float32).T[:, fsl]),
            "wkT": np.ascontiguousarray(np.asarray(wk, np.float32).T[:, fsl]),
            "wvT": np.ascontiguousarray(np.asarray(wv, np.float32).T[:, fsl]),
            "woT": woT,
            "cosT": cosT,
            "sinpmT": sinpm,
            "idf": idf,
            "idb": idb,
        })
    return in_maps


def run(cfg, x, wq, wk, wv, wo, **kw):
    in_maps = make_in_maps(cfg, x, wq, wk, wv, wo)
    nc = _get_nc(cfg)
    res = run_bass_kernel_spmd(nc, in_maps, list(range(NCORES)), **kw)
    y = np.concatenate([res.results[c]["y"] for c in range(NCORES)], 0)
    return y.reshape(cfg.B, cfg.T, cfg.D)


def kernel(x, wq, wk, wv, wo):
    return run(Cfg(), x, wq, wk, wv, wo)


if __name__ == "__main__":
    cfg = Cfg()
    rng = np.random.default_rng(0)
    x = rng.standard_normal((cfg.B, cfg.T, cfg.D)).astype(np.float32)
    ws = [(rng.standard_normal((cfg.D, cfg.D)) * 0.02).astype(np.float32)
          for _ in range(4)]
    y = kernel(x, *ws)
    print("out", y.shape, y.dtype, float(np.abs(y).max()))



# revision 19
# speedup vs baseline: 52.9789x; 1.1722x over previous
"""BitAttention (ternary-weight attention with int4/topk-int8 activation quant)
on 8 Trainium2 NeuronCores.

Sharding: tensor-parallel over heads for qkv-proj + SDPA (heads/8 per core),
AllToAll re-shard to token-parallel for the topk+int8 o-projection.

Numerics: quantized values are exact small integers, so qkv/o projections run
as exact integer arithmetic in bf16 matmuls (fp32 PSUM accumulate). Attention
(rope'd q/k real-valued) runs in fp32 matmuls. Softmax exp on ACT. Top-k
per-row threshold found by binary search on the |value| axis; per-token scales
folded into rope tables / exp bias / output scaling.
"""
import math
import numpy as np
import ml_dtypes

# ---------------------------------------------------------------------------
# TileContext patches for this walrus build (single sem-wait per instruction).
# ---------------------------------------------------------------------------
import re as _re
import concourse.mybir as mybir
import concourse.bass as bass
import concourse.tile as tile
from concourse.tile import TileContext, ScopedClock, VectorClock
from concourse.bass_utils import run_bass_kernel_spmd

_carrier_seq = [0]
_orig_add_instruction = TileContext._add_instruction


def _patched_add_instruction(self, inst):
    si = inst.sync_info
    if si is not None and si.on_wait is not None and len(si.on_wait) > 1:
        waits = list(si.on_wait)
        for w in waits[:-1]:
            _carrier_seq[0] += 1
            carrier = mybir.InstEventSemaphore(
                name=f"waitc_{_carrier_seq[0]}_{inst.name}",
                engine=inst.engine,
                ins=[],
                outs=[],
                sync_info=mybir.SyncInfo(on_wait=[w], on_update=[]),
            )
            _orig_add_instruction(self, carrier)
        si.on_wait = [waits[-1]]
        inst.sync_info = si
    _orig_add_instruction(self, inst)


def _clock_ticks(clock):
    m = _re.match(r"VectorClock\((\[.*\])\)", repr(clock))
    return eval(m.group(1))


def _patched_drain_and_barrier(self, tick_clock, wait_clock):
    nc = self.nc
    ticks = _clock_ticks(tick_clock.global_clock)
    n = len(ticks)
    for i, t in enumerate(ticks):
        if t > 0:
            d = nc.sync.drain()
            vci = VectorClock([t if j == i else 0 for j in range(n)])
            wait_clock.add_sem_waits(d.ins, ScopedClock({None: vci}))
    nc.sync.drain()
    nc.all_engine_barrier()
    assert self.sems is not None
    popped = nc._tile_sem_poison_stack.pop()
    assert popped is self._sem_poison
    nc.clear_and_free_semaphores(list(self.sems.allocated().values()))
    nc.all_engine_barrier()


TileContext._add_instruction = _patched_add_instruction
TileContext._drain_and_barrier = _patched_drain_and_barrier

# ---------------------------------------------------------------------------

F32 = mybir.dt.float32
BF16 = mybir.dt.bfloat16
F16 = mybir.dt.float16
AF = mybir.ActivationFunctionType
ALU = mybir.AluOpType
AX = mybir.AxisListType
MAGIC = 1.5 * 2.0 ** 23
EPS = 1e-5
THETA = 10000.0
TOPK_RATIO = 0.55
NCORES = 8


class Cfg:
    def __init__(self, B=2, T=2048, D=2048, H=16, HD=128, chunk=256, qchunk=512,
                 search_iters=26, no_collectives=False, stop_after=''):
        self.B, self.T, self.D, self.H, self.HD = B, T, D, H, HD
        self.NT = B * T
        self.HPC = H // NCORES            # heads per core
        self.FS = self.HPC * HD           # feature slice per core
        self.chunk = chunk                # phase-A token chunk
        self.qchunk = qchunk              # attention q chunk
        self.TPC = self.NT // NCORES      # tokens per core in phase C
        self.K = max(1, int(TOPK_RATIO * D))
        self.search_iters = search_iters
        self.no_collectives = no_collectives
        self.stop_after = stop_after
        assert self.NT % 128 == 0 and D % 512 == 0 and HD % 2 == 0
        assert T % qchunk == 0 and self.NT % chunk == 0 and chunk % 128 == 0
        assert self.TPC % 128 == 0 and HD <= 128 and self.FS % 128 == 0
        assert D == H * HD


def rope_tables(cfg):
    hd, T = cfg.HD, cfg.T
    inv = 1.0 / THETA ** (np.arange(0, hd, 2, dtype=np.float32) / hd)
    freqs = np.arange(T, dtype=np.float32)[:, None] * inv[None, :]
    emb = np.concatenate([freqs, freqs], axis=1)          # (T, hd)
    cos = np.cos(emb).astype(np.float32)
    sin = np.sin(emb).astype(np.float32)
    cosT = np.concatenate([cos] * cfg.B, 0).T.copy()      # (hd, NT)
    sinT = np.concatenate([sin] * cfg.B, 0).T.copy()
    sin_pm = sinT.copy()
    sin_pm[: hd // 2] = -sin_pm[: hd // 2]                # rotate-half signs
    return np.ascontiguousarray(cosT), np.ascontiguousarray(sin_pm)


def build(cfg: Cfg):
    nc = bass.Bass("TRN2", target_bir_lowering=False, debug=False,
                   num_devices=NCORES)
    NT, D, HD, FS, TPC = cfg.NT, cfg.D, cfg.HD, cfg.FS, cfg.TPC

    x_d = nc.dram_tensor("x", [NT, D], F32, kind="ExternalInput")
    wqT_d = nc.dram_tensor("wqT", [D, FS], F32, kind="ExternalInput")
    wkT_d = nc.dram_tensor("wkT", [D, FS], F32, kind="ExternalInput")
    wvT_d = nc.dram_tensor("wvT", [D, FS], F32, kind="ExternalInput")
    woT_d = nc.dram_tensor("woT", [D, D], F32, kind="ExternalInput")
    cos_d = nc.dram_tensor("cosT", [HD, NT], F32, kind="ExternalInput")
    sin_d = nc.dram_tensor("sinpmT", [HD, NT], F32, kind="ExternalInput")
    idf_d = nc.dram_tensor("idf", [128, 128], F32, kind="ExternalInput")
    idb_d = nc.dram_tensor("idb", [128, 128], BF16, kind="ExternalInput")
    y_d = nc.dram_tensor("y", [TPC, D], F32, kind="ExternalOutput")

    with TileContext(nc, pool_alloc_mode="queue") as tc:
        _body(nc, tc, cfg, x_d, wqT_d, wkT_d, wvT_d, woT_d, cos_d, sin_d,
              idf_d, idb_d, y_d)
    return nc


def _body(nc, tc, cfg, x_d, wqT_d, wkT_d, wvT_d, woT_d, cos_d, sin_d,
          idf_d, idb_d, y_d):
    NT, D, HD, HPC, FS = cfg.NT, cfg.D, cfg.HD, cfg.HPC, cfg.FS
    NTT, NDT = NT // 128, D // 128
    CH = cfg.chunk
    NCH, CTT = NT // CH, CH // 128
    QC, KT, NQC = cfg.qchunk, cfg.T // 128, cfg.T // cfg.qchunk
    TPC, FTQ = cfg.TPC, FS // 128
    SQRT_SCALE = float(1.0 / math.sqrt(HD))
    HH = HD // 2

    with tc.tile_pool(name="persist", bufs=1) as pp, \
         tc.tile_pool(name="dram", bufs=1, space="DRAM") as dramp:
        idf = pp.tile([128, 128], F32)
        nc.sync.dma_start(idf[:], idf_d[:])
        idb = pp.tile([128, 128], BF16)
        nc.sync.dma_start(idb[:], idb_d[:])
        ones_col = pp.tile([128, 1], F32)        # partition sums (lhsT)
        nc.gpsimd.memset(ones_col[:], 1.0)
        one1 = pp.tile([1, 1], F32)              # 1x1 for outer-product transposes
        nc.gpsimd.memset(one1[:], 1.0)
        ones_row = pp.tile([1, 128], F32)        # partition broadcast (lhsT)
        nc.gpsimd.memset(ones_row[:], 1.0)
        # per token-tile columns (tokens on partitions)
        inv_sx = pp.tile([128, NTT], F32)
        ln_sv = pp.tile([128, NTT], F32)
        rinv_sv = pp.tile([128, NTT], F32)
        ws_s = pp.tile([128, 4], F32)   # bcast weight scales s_w (q,k,v,o)
        ws_r = pp.tile([128, 4], F32)   # bcast 1/(s_w+eps)
        sob = pp.tile([128, 2], F32)    # bcast (s_wo, 1/(s_wo+eps))
        a2a_in = dramp.tile([NT, FS], F32)
        a2a_out = dramp.tile([NT, FS], F32)

        with tc.tile_pool(name="pAB", bufs=1) as pab:
            # roped q/k [FS, NT] fp32; integer v in token layout (fp16, exact)
            qT = [pab.tile([128, NT], F32, tag=f"qT{i}", name=f"qT{i}") for i in range(FTQ)]
            kT = [pab.tile([128, NT], F32, tag=f"kT{i}", name=f"kT{i}") for i in range(FTQ)]
            vtok = [pab.tile([128, FS], F16, tag=f"vtok{i}", name=f"vtok{i}") for i in range(NTT)]

            with tc.tile_pool(name="pW", bufs=1) as pw:
                # ternarized weight slices (bf16 ints), persist through phase A
                wqt = [pw.tile([128, FS], BF16, tag=f"wqt{i}", name=f"wqt{i}") for i in range(NDT)]
                wkt = [pw.tile([128, FS], BF16, tag=f"wkt{i}", name=f"wkt{i}") for i in range(NDT)]
                wvt = [pw.tile([128, FS], BF16, tag=f"wvt{i}", name=f"wvt{i}") for i in range(NDT)]
                _phase_w(nc, tc, cfg, dramp, wqT_d, wkT_d, wvT_d, woT_d,
                         ones_col, ones_row, ws_s, ws_r, wqt, wkt, wvt)
                if cfg.stop_after == 'W':
                    return
                _phase_a(nc, tc, cfg, x_d, cos_d, sin_d, idf, idb, ones_row,
                         ws_s, inv_sx, ln_sv, rinv_sv, wqt, wkt, wvt,
                         qT, kT, vtok)
            if cfg.stop_after == 'A':
                return
            wom = _WoMean(nc, tc, cfg, woT_d, ones_col, ones_row, sob)
            _phase_b(nc, tc, cfg, idf, ones_col, one1, ln_sv, rinv_sv,
                     qT, kT, vtok, a2a_in, wom)
            wom.finish()
        if cfg.stop_after == 'B':
            return

        if cfg.no_collectives:
            nc.sync.dma_start(a2a_out[:], a2a_in[:])
        else:
            nc.gpsimd.collective_compute(
                "AllToAll", ALU.bypass, replica_groups=[list(range(NCORES))],
                ins=[a2a_in[:].opt()], outs=[a2a_out[:].opt()])
        _phase_c(nc, tc, cfg, woT_d, idb, ws_s, ws_r, ones_col, ones_row, sob, a2a_out, y_d)


def _phase_w(nc, tc, cfg, dramp, wqT_d, wkT_d, wvT_d, woT_d, ones_col,
             ones_row, ws_s, ws_r, wqt, wkt, wvt):
    D, FS = cfg.D, cfg.FS
    NDT = D // 128
    with tc.tile_pool(name="ph_w", bufs=1) as wp, \
         tc.tile_pool(name="ph_w_ps", bufs=2, space="PSUM") as wps, \
         tc.tile_pool(name="ph_w1", bufs=1) as wp1:
        partials = wp1.tile([1, 4], F32)
        wraw = {}
        for j, wd in enumerate([wqT_d, wkT_d, wvT_d]):
            acc = wp1.tile([128, 1], F32, tag=f"wacc{j}", name=f"wacc{j}")
            nc.gpsimd.memset(acc[:], 0.0)
            for dt in range(NDT):
                t = wp.tile([128, FS], F32, tag=f"wld_{j}_{dt}",
                            name=f"wld_{j}_{dt}")
                nc.sync.dma_start(t[:], wd[dt * 128:(dt + 1) * 128, :])
                wraw[(j, dt)] = t
                r = wp1.tile([128, 1], F32, tag="wred")
                nc.vector.tensor_reduce(r[:], t[:], axis=AX.X, op=ALU.add,
                                        apply_absolute_value=True)
                nc.vector.tensor_tensor(acc[:], acc[:], r[:], op=ALU.add)
            ps = wps.tile([1, 1], F32, tag="w_ps1")
            nc.tensor.matmul(ps[:], acc[:], ones_col[:, 0:1], start=True,
                             stop=True)
            nc.vector.tensor_copy(partials[:, j:j + 1], ps[:])
        nc.gpsimd.memset(partials[:, 3:4], 0.0)
        ar_in = dramp.tile([1, 4], F32)
        ar_out = dramp.tile([1, 4], F32, addr_space="Shared")
        nc.sync.dma_start(ar_in[:], partials[:])
        if cfg.no_collectives:
            nc.sync.dma_start(ar_out[:], ar_in[:])
        else:
            nc.gpsimd.collective_compute(
                "AllReduce", ALU.add, replica_groups=[list(range(NCORES))],
                ins=[ar_in[:].opt()], outs=[ar_out[:].opt()])
        sums = wp1.tile([1, 4], F32)
        nc.sync.dma_start(sums[:], ar_out[:])
        s_row = wp1.tile([1, 4], F32)
        nc.vector.tensor_scalar(s_row[:], sums[:], 1.0 / (float(D) * float(D)),
                                None, op0=ALU.mult)
        r_row = wp1.tile([1, 4], F32)
        nc.vector.tensor_scalar(r_row[:], s_row[:], EPS, None, op0=ALU.add)
        nc.vector.reciprocal(r_row[:], r_row[:])
        ps_b = wps.tile([128, 4], F32, tag="w_psb")
        nc.tensor.matmul(ps_b[:], ones_row[:], s_row[:], start=True, stop=True)
        nc.scalar.copy(ws_s[:], ps_b[:])
        ps_b2 = wps.tile([128, 4], F32, tag="w_psb")
        nc.tensor.matmul(ps_b2[:], ones_row[:], r_row[:], start=True,
                         stop=True)
        nc.scalar.copy(ws_r[:], ps_b2[:])
        # ternarize in place from the resident raw tiles
        for j, dst in enumerate([wqt, wkt, wvt]):
            for dt in range(NDT):
                t = wraw[(j, dt)]
                nc.vector.tensor_scalar(t[:], t[:], ws_r[:, j:j + 1], MAGIC,
                                        op0=ALU.mult, op1=ALU.add)
                nc.vector.tensor_scalar(t[:], t[:], MAGIC, -1.0,
                                        op0=ALU.subtract, op1=ALU.max)
                nc.vector.tensor_scalar(dst[dt][:], t[:], 1.0, None,
                                        op0=ALU.min)


def _phase_a(nc, tc, cfg, x_d, cos_d, sin_d, idf, idb, ones_row, ws_s,
             inv_sx, ln_sv, rinv_sv, wqt, wkt, wvt, qT, kT, vtok):
    D, HD, FS = cfg.D, cfg.HD, cfg.FS
    NDT = D // 128
    CH = cfg.chunk
    NCH, CTT = cfg.NT // CH, CH // 128
    FTQ = FS // 128
    HH = HD // 2
    with tc.tile_pool(name="ph_a", bufs=2) as ap, \
         tc.tile_pool(name="ph_a3", bufs=3) as ap3, \
         tc.tile_pool(name="ph_a_ps", bufs=3, space="PSUM") as aps, \
         tc.tile_pool(name="ph_a_ps3", bufs=4, space="PSUM") as aps3:
        for ch in range(NCH):
            t0 = ch * CH
            xq = [None] * CTT
            for j in range(CTT):
                tt = t0 // 128 + j
                xt = ap.tile([128, D], F32, tag="xload")
                nc.sync.dma_start(xt[:], x_d[tt * 128:(tt + 1) * 128, :])
                m = ap.tile([128, 1], F32, tag="xm")
                nc.vector.tensor_reduce(m[:], xt[:], axis=AX.X, op=ALU.max,
                                        apply_absolute_value=True)
                nc.vector.tensor_scalar(m[:], m[:], EPS, None, op0=ALU.max)
                nc.vector.tensor_scalar(inv_sx[:, tt:tt + 1], m[:], 1.0 / 7.0,
                                        None, op0=ALU.mult)
                sx = ap.tile([128, 1], F32, tag="xs")
                nc.vector.reciprocal(sx[:], m[:])
                nc.vector.tensor_scalar(sx[:], sx[:], 7.0, None, op0=ALU.mult)
                sv = ap.tile([128, 1], F32, tag="xsv")
                nc.vector.tensor_tensor(sv[:], inv_sx[:, tt:tt + 1],
                                        ws_s[:, 2:3], op=ALU.mult)
                nc.scalar.activation(ln_sv[:, tt:tt + 1], sv[:], AF.Ln)
                nc.vector.reciprocal(rinv_sv[:, tt:tt + 1], sv[:])
                xqj = ap.tile([128, D], BF16, tag="xq")
                tmp = ap.tile([128, D], F32, tag="xtmp")
                nc.vector.tensor_scalar(tmp[:], xt[:], sx[:], MAGIC,
                                        op0=ALU.mult, op1=ALU.add)
                nc.vector.tensor_scalar(xqj[:], tmp[:], MAGIC, None,
                                        op0=ALU.subtract)
                xq[j] = xqj
            # transpose xq -> xqT tiles [128 d, CH] (bf16 ints)
            xqT = [None] * NDT
            for dt in range(NDT):
                pst = aps.tile([128, CH], BF16, tag="ps_misc")
                for j in range(CTT):
                    nc.tensor.transpose(pst[:, j * 128:(j + 1) * 128],
                                        xq[j][:, dt * 128:(dt + 1) * 128],
                                        idb[:])
                xqT[dt] = ap.tile([128, CH], BF16, tag=f"xqT{dt}", name=f"xqT{dt}")
                nc.scalar.copy(xqT[dt][:], pst[:])
            # scaled rope tables for this chunk
            cos_c = ap.tile([HD, CH], F32, tag="cos_c")
            nc.sync.dma_start(cos_c[:], cos_d[:, t0:t0 + CH])
            sin_c = ap.tile([HD, CH], F32, tag="sin_c")
            nc.sync.dma_start(sin_c[:], sin_d[:, t0:t0 + CH])
            tabs = {}
            for (wj, nm) in ((0, 'q'), (1, 'k')):
                colp = ap.tile([128, CTT], F32, tag="colp")
                nc.vector.tensor_scalar(colp[:],
                                        inv_sx[:, t0 // 128:t0 // 128 + CTT],
                                        ws_s[:, wj:wj + 1], None, op0=ALU.mult)
                pst = aps.tile([128, 128], F32, tag="ps_misc")
                nc.tensor.transpose(pst[:CTT, :], colp[:], idf[:])
                srow_t = ap.tile([CTT, 128], F32, tag="srowt")
                nc.scalar.copy(srow_t[:], pst[:CTT, :])
                srow = ap.tile([1, CH], F32, tag="srow")
                nc.sync.dma_start(srow[:], srow_t[:])
                psb = aps.tile([HD, CH], F32, tag="ps_misc")
                nc.tensor.matmul(psb[:], ones_row[:, :HD], srow[:],
                                 start=True, stop=True)
                sb = ap.tile([HD, CH], F32, tag=f"sb_{nm}", name=f"sb_{nm}")
                nc.scalar.copy(sb[:], psb[:])
                tc_t = ap.tile([HD, CH], F32, tag=f"tc_{nm}", name=f"tc_{nm}")
                nc.vector.tensor_tensor(tc_t[:], cos_c[:], sb[:], op=ALU.mult)
                ts_t = ap.tile([HD, CH], F32, tag=f"ts_{nm}", name=f"ts_{nm}")
                nc.vector.tensor_tensor(ts_t[:], sin_c[:], sb[:], op=ALU.mult)
                tabs[nm] = (tc_t, ts_t)
            # projections + rope drains
            for wt, nm, dstT in ((wqt, 'q', qT), (wkt, 'k', kT)):
                table_c, table_s = tabs[nm]
                for ft in range(FTQ):
                    ps = aps3.tile([128, CH], F32, tag="ps_proj")
                    for dt in range(NDT):
                        nc.tensor.matmul(ps[:],
                                         wt[dt][:, ft * 128:(ft + 1) * 128],
                                         xqT[dt][:], start=(dt == 0),
                                         stop=(dt == NDT - 1))
                    dtile = dstT[ft]
                    for hb in range(128 // HD):
                        fo = hb * HD
                        t1 = ap3.tile([HD, CH], F32, tag="ropet1")
                        nc.vector.tensor_tensor(t1[:], ps[fo:fo + HD, :],
                                                table_c[:], op=ALU.mult)
                        t2 = ap3.tile([HD, CH], F32, tag="ropet2")
                        nc.vector.tensor_tensor(t2[:HH, :],
                                                ps[fo + HH:fo + HD, :],
                                                table_s[:HH, :], op=ALU.mult)
                        nc.vector.tensor_tensor(t2[HH:, :], ps[fo:fo + HH, :],
                                                table_s[HH:, :], op=ALU.mult)
                        nc.vector.tensor_tensor(dtile[fo:fo + HD, t0:t0 + CH],
                                                t1[:], t2[:], op=ALU.add)
            # v: integer result (exact in fp16), transpose to token layout
            for ft in range(FTQ):
                ps = aps3.tile([128, CH], F32, tag="ps_proj")
                for dt in range(NDT):
                    nc.tensor.matmul(ps[:], wvt[dt][:, ft * 128:(ft + 1) * 128],
                                     xqT[dt][:], start=(dt == 0),
                                     stop=(dt == NDT - 1))
                vtmp = ap3.tile([128, CH], F32, tag="vtmp")
                nc.scalar.copy(vtmp[:], ps[:])
                for j in range(CTT):
                    tt = t0 // 128 + j
                    pst = aps.tile([128, 128], F32, tag="ps_misc")
                    nc.tensor.transpose(pst[:], vtmp[:, j * 128:(j + 1) * 128],
                                        idf[:])
                    nc.scalar.copy(vtok[tt][:, ft * 128:(ft + 1) * 128],
                                   pst[:])


class _WoMean:
    """mean|wo| pass, emitted one tile per attention iteration so the DVE/DMA
    work interleaves with phase B instead of serializing before it."""

    def __init__(self, nc, tc, cfg, woT_d, ones_col, ones_row, sob):
        self.nc, self.tc, self.cfg = nc, tc, cfg
        self.woT_d, self.ones_col, self.ones_row, self.sob = (
            woT_d, ones_col, ones_row, sob)
        self.NDT = cfg.D // 128
        self._cms = [tc.tile_pool(name="pwo", bufs=2),
                     tc.tile_pool(name="pwo1", bufs=1),
                     tc.tile_pool(name="pwo_ps", bufs=1, space="PSUM")]
        self.wp = self._cms[0].__enter__()
        self.wp1 = self._cms[1].__enter__()
        self.wps = self._cms[2].__enter__()
        self.acc = self.wp1.tile([128, 1], F32, name="wo_acc")
        nc.gpsimd.memset(self.acc[:], 0.0)
        self.done = 0

    def step(self):
        if self.done >= self.NDT:
            return
        nc, D = self.nc, self.cfg.D
        dt = self.done
        self.done += 1
        t = self.wp.tile([128, D], F32, tag="wo_ld", name="wo_ld")
        nc.sync.dma_start(t[:], self.woT_d[dt * 128:(dt + 1) * 128, :])
        r = self.wp1.tile([128, 1], F32, tag="wo_red", name="wo_red")
        nc.vector.tensor_reduce(r[:], t[:], axis=AX.X, op=ALU.add,
                                apply_absolute_value=True)
        nc.vector.tensor_tensor(self.acc[:], self.acc[:], r[:], op=ALU.add)

    def finish(self):
        while self.done < self.NDT:
            self.step()
        nc, D = self.nc, self.cfg.D
        pss = self.wps.tile([1, 1], F32, tag="wo_ps", name="wo_ps")
        nc.tensor.matmul(pss[:], self.acc[:], self.ones_col[:, 0:1],
                         start=True, stop=True)
        so_s = self.wp1.tile([1, 1], F32, name="so_s")
        nc.vector.tensor_scalar(so_s[:], pss[:], 1.0 / (float(D) * float(D)),
                                None, op0=ALU.mult)
        so_r = self.wp1.tile([1, 1], F32, name="so_r")
        nc.vector.tensor_scalar(so_r[:], so_s[:], EPS, None, op0=ALU.add)
        nc.vector.reciprocal(so_r[:], so_r[:])
        sr2 = self.wp1.tile([1, 2], F32, name="sr2")
        nc.vector.tensor_copy(sr2[:, 0:1], so_s[:])
        nc.vector.tensor_copy(sr2[:, 1:2], so_r[:])
        psb = self.wps.tile([128, 2], F32, tag="wo_ps", name="wo_psb")
        nc.tensor.matmul(psb[:], self.ones_row[:], sr2[:], start=True,
                         stop=True)
        nc.scalar.copy(self.sob[:], psb[:])
        for cm in reversed(self._cms):
            cm.__exit__(None, None, None)


def _phase_b(nc, tc, cfg, idf, ones_col, one1, ln_sv, rinv_sv,
             qT, kT, vtok, a2a_in, wom=None):
    """Attention. Scores in fp32 matmuls (precision-critical); pT =
    exp(score*scale + ln_sv) emitted in fp16 (2.4e-4 relative, inside the
    downstream quantization noise budget); attn@v runs as fp16 matmuls
    against the exact integer v. Denominators accumulate on DVE, partition
    -summed via a ones matmul, and applied after the output transpose
    where they are per-partition scalars."""
    HD, HPC = cfg.HD, cfg.HPC
    QC, KT, NQC = cfg.qchunk, cfg.T // 128, cfg.T // cfg.qchunk
    NJ = QC // 128
    scale = SQRT_SCALE_OF(cfg)
    with tc.tile_pool(name="ph_b_p", bufs=3) as bp, \
         tc.tile_pool(name="ph_b_p2", bufs=2) as bp2, \
         tc.tile_pool(name="ph_b_ps_sc", bufs=2, space="PSUM") as ps_sc, \
         tc.tile_pool(name="ph_b_ps_o", bufs=2, space="PSUM") as ps_o, \
         tc.tile_pool(name="ph_b_ps_d", bufs=1, space="PSUM") as ps_d, \
         tc.tile_pool(name="ph_b_ps_m", bufs=2, space="PSUM") as ps_m:
        for b in range(cfg.B):
            for hh in range(HPC):
                fo = hh * HD
                ftile, fin = fo // 128, fo % 128
                for qc in range(NQC):
                    q0 = b * cfg.T + qc * QC
                    outp = ps_o.tile([HD, QC], F32, tag="outp")
                    den = bp2.tile([128, QC], F32, tag="den")
                    if wom is not None:
                        wom.step()

                    # software pipeline: scores(kt+1) issue on the PE
                    # before attn@v(kt), which waits on exp(kt)
                    def emit_scores(kt):
                        k0 = b * cfg.T + kt * 128
                        ktt = (b * cfg.T) // 128 + kt
                        ssc = ps_sc.tile([128, QC], F32, tag="ssc")
                        nc.tensor.matmul(
                            ssc[:], kT[ftile][fin:fin + HD, k0:k0 + 128],
                            qT[ftile][fin:fin + HD, q0:q0 + QC],
                            start=True, stop=True)
                        pt = bp.tile([128, QC], F16, tag="pt")
                        nc.scalar.activation(pt[:], ssc[:], AF.Exp,
                                             bias=ln_sv[:, ktt:ktt + 1],
                                             scale=scale)
                        return pt

                    def emit_av(kt, pt):
                        ktt = (b * cfg.T) // 128 + kt
                        nc.tensor.matmul(outp[:], vtok[ktt][:, fo:fo + HD],
                                         pt[:], start=(kt == 0),
                                         stop=(kt == KT - 1))
                        if kt == 0:
                            nc.vector.tensor_scalar(
                                den[:], pt[:], rinv_sv[:, ktt:ktt + 1],
                                None, op0=ALU.mult)
                        else:
                            nc.vector.scalar_tensor_tensor(
                                den[:], in0=pt[:],
                                scalar=rinv_sv[:, ktt:ktt + 1], in1=den[:],
                                op0=ALU.mult, op1=ALU.add)

                    prev = emit_scores(0)
                    for kt in range(1, KT):
                        cur = emit_scores(kt)
                        emit_av(kt - 1, prev)
                        prev = cur
                    emit_av(KT - 1, prev)
                    # denominators: partition sum, transpose to q-major
                    dps = ps_d.tile([1, QC], F32, tag="dps")
                    nc.tensor.matmul(dps[:], ones_col[:], den[:],
                                     start=True, stop=True)
                    dsb = bp2.tile([1, QC], F32, tag="dsb")
                    nc.scalar.copy(dsb[:], dps[:])
                    dpsT = ps_m.tile([128, 128], F32, tag="pmisc")
                    for j in range(NJ):
                        nc.tensor.matmul(dpsT[:, j:j + 1],
                                         dsb[:, j * 128:(j + 1) * 128],
                                         one1[:], start=True, stop=True)
                    rden = bp2.tile([128, NJ], F32, tag="rden")
                    nc.vector.reciprocal(rden[:], dpsT[:, :NJ])
                    # output: sbuf copy, transpose to token-major, normalize
                    osb = bp2.tile([HD, QC], F32, tag="osb")
                    nc.scalar.copy(osb[:], outp[:])
                    for j in range(NJ):
                        pst = ps_m.tile([128, HD], F32, tag="pmisc")
                        nc.tensor.transpose(pst[:],
                                            osb[:, j * 128:(j + 1) * 128],
                                            idf[:])
                        stg = bp2.tile([128, HD], F32, tag="stg")
                        nc.vector.tensor_scalar(stg[:], pst[:],
                                                rden[:, j:j + 1], None,
                                                op0=ALU.mult)
                        r0 = q0 + j * 128
                        nc.sync.dma_start(a2a_in[r0:r0 + 128, fo:fo + HD],
                                          stg[:])


def SQRT_SCALE_OF(cfg):
    return float(1.0 / math.sqrt(cfg.HD))


def _phase_c(nc, tc, cfg, woT_d, idb, ws_s, ws_r, ones_col, ones_row, sob, a2a_out, y_d):
    D, TPC = cfg.D, cfg.TPC
    NDT = D // 128
    NTC = TPC // 128
    NFC = D // 512
    with tc.tile_pool(name="pc0", bufs=1) as pc0:
        m8 = pc0.tile([128, NTC], F32)
        lo = pc0.tile([128, NTC], F32)
        s8 = pc0.tile([128, NTC], F32)
        x8 = [pc0.tile([128, D], BF16, tag=f"x8_{j}", name=f"x8_{j}")
              for j in range(NTC)]
        # --- C1: load, abs, threshold search, int8 quant + mask ---
        with tc.tile_pool(name="pc1", bufs=1) as cp1, \
             tc.tile_pool(name="pc1w", bufs=3) as cpw:
            a2a_v = a2a_out[:].rearrange("(s t) f -> t s f", s=NCORES)
            at, absa = [], []
            for j in range(NTC):
                t = cp1.tile([128, D], F32, tag=f"at{j}", name=f"at{j}")
                nc.sync.dma_start(t[:].rearrange("p (s f) -> p s f", s=NCORES),
                                  a2a_v[j * 128:(j + 1) * 128])
                at.append(t)
                ab = cp1.tile([128, D], F32, tag=f"ab{j}", name=f"ab{j}")
                nc.scalar.activation(ab[:], t[:], AF.Abs)
                absa.append(ab)
                nc.vector.tensor_reduce(m8[:, j:j + 1], ab[:], axis=AX.X,
                                        op=ALU.max)
            nc.vector.tensor_scalar(m8[:], m8[:], EPS, None, op0=ALU.max)
            # binary search for the k-th largest |a| per row
            nc.gpsimd.memset(lo[:], 0.0)
            hi = cp1.tile([128, NTC], F32)
            nc.vector.tensor_scalar(hi[:], m8[:], 1.0001, None, op0=ALU.mult)
            mid = cp1.tile([128, NTC], F32)
            nmid = cp1.tile([128, NTC], F32)
            cnt = cp1.tile([128, NTC], F32)
            ge = cp1.tile([128, NTC], F32)
            dif = cp1.tile([128, NTC], F32)
            junk = cp1.tile([128, D], F32)
            junka = cp1.tile([128, D], F32)
            # first iters: upper half of the token tiles counted on ACT via
            # Sign+accum (acc = #above - #below); later iters all on DVE
            # (exact >= semantics near convergence).
            nh = NTC // 2
            act_iters = max(0, cfg.search_iters - 10) if nh else 0
            for it in range(cfg.search_iters):
                nc.vector.tensor_tensor(mid[:], lo[:], hi[:], op=ALU.add)
                nc.vector.tensor_scalar(mid[:], mid[:], 0.5, None, op0=ALU.mult)
                use_act = it < act_iters
                if use_act:
                    nc.vector.tensor_scalar(nmid[:], mid[:], -1.0, None,
                                            op0=ALU.mult)
                for j in range(NTC):
                    if use_act and j >= NTC - nh:
                        nc.scalar.activation(junka[:], absa[j][:], AF.Sign,
                                             bias=nmid[:, j:j + 1],
                                             accum_out=cnt[:, j:j + 1])
                    else:
                        nc.vector.tensor_scalar(junk[:], absa[j][:],
                                                mid[:, j:j + 1], None,
                                                op0=ALU.is_ge, op1=ALU.add,
                                                accum_out=cnt[:, j:j + 1])
                if use_act:
                    nc.vector.tensor_scalar(ge[:, :NTC - nh],
                                            cnt[:, :NTC - nh], float(cfg.K),
                                            None, op0=ALU.is_ge)
                    nc.vector.tensor_scalar(ge[:, NTC - nh:],
                                            cnt[:, NTC - nh:],
                                            float(2 * cfg.K - D), None,
                                            op0=ALU.is_ge)
                else:
                    nc.vector.tensor_scalar(ge[:], cnt[:], float(cfg.K), None,
                                            op0=ALU.is_ge)
                nc.vector.tensor_tensor(dif[:], mid[:], lo[:], op=ALU.subtract)
                nc.vector.tensor_tensor(dif[:], ge[:], dif[:], op=ALU.mult)
                nc.vector.tensor_tensor(lo[:], lo[:], dif[:], op=ALU.add)
                nc.vector.tensor_tensor(dif[:], hi[:], mid[:], op=ALU.subtract)
                nc.vector.tensor_tensor(dif[:], ge[:], dif[:], op=ALU.mult)
                nc.vector.tensor_tensor(hi[:], mid[:], dif[:], op=ALU.add)
            # quantize: x8 = round(a * s8) * (|a| >= lo), s8 = 127/m8
            nc.vector.reciprocal(s8[:], m8[:])
            nc.vector.tensor_scalar(s8[:], s8[:], 127.0, None, op0=ALU.mult)
            for j in range(NTC):
                tmp = cpw.tile([128, D], F32, tag="c_tmp")
                nc.vector.tensor_scalar(tmp[:], at[j][:], s8[:, j:j + 1],
                                        MAGIC, op0=ALU.mult, op1=ALU.add)
                nc.vector.tensor_scalar(tmp[:], tmp[:], MAGIC, None,
                                        op0=ALU.subtract)
                msk = cpw.tile([128, D], F32, tag="c_msk")
                nc.vector.tensor_scalar(msk[:], absa[j][:], lo[:, j:j + 1],
                                        None, op0=ALU.is_ge)
                nc.vector.tensor_tensor(x8[j][:], tmp[:], msk[:], op=ALU.mult)
        # --- C2: transpose x8, ternarize woT, matmul, scale, store ---
        with tc.tile_pool(name="pc2", bufs=1) as cp2, \
             tc.tile_pool(name="pc2w", bufs=3) as cw2, \
             tc.tile_pool(name="pc2_ps", bufs=3, space="PSUM") as cps:
            x8T = []
            for dt in range(NDT):
                pst = cps.tile([128, TPC], BF16, tag="c_pstr")
                for j in range(NTC):
                    nc.tensor.transpose(pst[:, j * 128:(j + 1) * 128],
                                        x8[j][:, dt * 128:(dt + 1) * 128],
                                        idb[:])
                t = cp2.tile([128, TPC], BF16, tag=f"x8T_{dt}",
                             name=f"x8T_{dt}")
                nc.scalar.copy(t[:], pst[:])
                x8T.append(t)
            wot = []
            for dt in range(NDT):
                t2 = cw2.tile([128, D], F32, tag="c_wo_t")
                nc.sync.dma_start(t2[:], woT_d[dt * 128:(dt + 1) * 128, :])
                nc.vector.tensor_scalar(t2[:], t2[:], sob[:, 1:2],
                                        MAGIC, op0=ALU.mult, op1=ALU.add)
                nc.vector.tensor_scalar(t2[:], t2[:], MAGIC, -1.0,
                                        op0=ALU.subtract, op1=ALU.max)
                tb = cp2.tile([128, D], BF16, tag=f"wot_{dt}",
                              name=f"wot_{dt}")
                nc.vector.tensor_scalar(tb[:], t2[:], 1.0, None, op0=ALU.min)
                wot.append(tb)
            # y = (x8 @ wot.T) * s_wo * m8 / 127
            ysc = cp2.tile([128, NTC], F32)
            nc.vector.tensor_scalar(ysc[:], m8[:], sob[:, 0:1], None,
                                    op0=ALU.mult)
            nc.vector.tensor_scalar(ysc[:], ysc[:], 1.0 / 127.0, None,
                                    op0=ALU.mult)
            for j in range(NTC):
                ysb = cw2.tile([128, D], F32, tag="c_y")
                for fc in range(NFC):
                    ps = cps.tile([128, 512], F32, tag="c_psy")
                    for dt in range(NDT):
                        nc.tensor.matmul(ps[:],
                                         x8T[dt][:, j * 128:(j + 1) * 128],
                                         wot[dt][:, fc * 512:(fc + 1) * 512],
                                         start=(dt == 0), stop=(dt == NDT - 1))
                    nc.vector.tensor_scalar(ysb[:, fc * 512:(fc + 1) * 512],
                                            ps[:], ysc[:, j:j + 1], None,
                                            op0=ALU.mult)
                nc.sync.dma_start(y_d[j * 128:(j + 1) * 128, :], ysb[:])


# ---------------------------------------------------------------------------
# Host-side driver
# ---------------------------------------------------------------------------
_CACHED = {}


def _get_nc(cfg):
    key = (cfg.B, cfg.T, cfg.D, cfg.H, cfg.HD, cfg.chunk, cfg.qchunk,
           cfg.search_iters, cfg.no_collectives, cfg.stop_after)
    if key not in _CACHED:
        _CACHED[key] = build(cfg)
    return _CACHED[key]


def make_in_maps(cfg, x, wq, wk, wv, wo):
    NT, D, FS = cfg.NT, cfg.D, cfg.FS
    x2 = np.ascontiguousarray(np.asarray(x, np.float32).reshape(NT, D))
    cosT, sinpm = rope_tables(cfg)
    idf = np.eye(128, dtype=np.float32)
    idb = idf.astype(ml_dtypes.bfloat16)
    woT = np.ascontiguousarray(np.asarray(wo, np.float32).T)
    in_maps = []
    for c in range(NCORES):
        fsl = slice(c * FS, (c + 1) * FS)
        in_maps.append({
            "x": x2,
            "wqT": np.ascontiguousarray(np.asarray(wq, np.float32).T[:, fsl]),
            "wkT": np.ascontiguousarray(np.asarray(wk, np.float32).T[:, fsl]),
            "wvT": np.ascontiguousarray(np.asarray(wv, np.float32).T[:, fsl]),
            "woT": woT,
            "cosT": cosT,
            "sinpmT": sinpm,
            "idf": idf,
            "idb": idb,
        })
    return in_maps


def run(cfg, x, wq, wk, wv, wo, **kw):
    in_maps = make_in_maps(cfg, x, wq, wk, wv, wo)
    nc = _get_nc(cfg)
    res = run_bass_kernel_spmd(nc, in_maps, list(range(NCORES)), **kw)
    y = np.concatenate([res.results[c]["y"] for c in range(NCORES)], 0)
    return y.reshape(cfg.B, cfg.T, cfg.D)


def kernel(x, wq, wk, wv, wo):
    return run(Cfg(), x, wq, wk, wv, wo)


if __name__ == "__main__":
    cfg = Cfg()
    rng = np.random.default_rng(0)
    x = rng.standard_normal((cfg.B, cfg.T, cfg.D)).astype(np.float32)
    ws = [(rng.standard_normal((cfg.D, cfg.D)) * 0.02).astype(np.float32)
          for _ in range(4)]
    y = kernel(x, *ws)
    print("out", y.shape, y.dtype, float(np.abs(y).max()))

